# revision 27
# baseline (speedup 1.0000x reference)
"""Trainium2 Bass kernel for nn_Decoder (dense transformer decoder, 2 layers).

Sharding (8 cores): core c = 2*b + r handles batch b, query-row half r.
- Attention (scores/softmax/AV, all heads) is split by query rows.
- K/V projections are computed for all rows (duplicated within the pair).
- Cross-attention V2 is computed for own rows then pair-AllGathered.
- BatchNorm statistics are 8-rank AllReduced (sums over all B*S rows).
- Layer boundary: pair-AllGather of the new input_multi halves.

v2 rewrite vs baseline:
- Scores matmuls are 64-contraction row-tiled (two heads run concurrently on
  PE half-arrays, uniform (64,128) tile mode within self-attention).
- Scores land in PSUM as bf16: one [128,2048] bank-pair holds a head-pair x
  2 key-chunks, consumed by a single EXP instruction.
- AV is row-tiled over kpos halves into two accumulator banks (P/Q) with the
  V-aug ones column producing denominators in row 64.
- Softmax denominators: DVE adds into a [12,512] tile, one batched
  reciprocal, a rearrange-DMA into [2, 6*512], then one tiny selector-matmul
  per head pair broadcasts 1/den across 128 partitions in PSUM.
- Cross-attention scores+exp depend only on encod, so they are issued to
  cover the BN1 AllReduce and the V2 AllGather. Cross denominators are
  layer-invariant and cached from layer 0.
- Layer boundary: AllGather of the pre-BN FFN residual runs concurrently
  with the BN-stats AllReduce; the affine is applied locally afterwards.

Layout: activations are feature-major ("X^T", [feat, token]) stored as
[128, chunk, tok] SBUF tiles (feature f = 128*chunk + partition).
V / V2 are token-major [tok, head*65] with a ones column appended per head
(V-aug) so softmax denominators fall out of the AV matmul as row 64.
All matmuls are bf16 x bf16; the residual stream and BN statistics are f32.
"""
import numpy as np
import ml_dtypes

B, S, D, H, VOCAB, NLAYERS = 4, 1024, 768, 12, 32000, 2
HD = D // H          # 64
R = S // 2           # 512 own rows per core
NC = 8
SCALE1 = 1.0 / float(np.sqrt(D))
SCALE2 = 1.0 / float(np.sqrt(HD))
INV_N = 1.0 / (B * S)
NKCH = S // 128      # 8 key chunks
NPAIR = H // 2       # 6 head pairs

BF = None
F32 = None

_CACHE = {}


def _pos_encoding():
    p = np.arange(S, dtype=np.float32)[:, None]
    i = np.arange(D // 2, dtype=np.float32)[None, :]
    ang = p / np.power(10000.0, 2.0 * i / D)
    return np.stack([np.sin(ang), np.cos(ang)], axis=-1).reshape(S, D).astype(np.float32)


def _fm(a):
    """[tok, feat] -> feature-major chunked [128, nchunk, tok]."""
    t, f = a.shape
    return np.ascontiguousarray(a.T.reshape(f // 128, 128, t).transpose(1, 0, 2))


def _wchunk(w):
    """[in, out] weight -> [128, nin, out] (stationary chunks)."""
    i, o = w.shape
    return np.ascontiguousarray(w.reshape(i // 128, 128, o).transpose(1, 0, 2))


def _col(v):
    """[768] -> [128, 6] feature-major columns."""
    return np.ascontiguousarray(v.reshape(6, 128).T)


def _bf16(a):
    return np.asarray(a, np.float32).astype(ml_dtypes.bfloat16)


def _build(taps=False):
    import concourse.bass as bass
    import concourse.mybir as mybir
    import concourse.tile as tile
    from concourse import bacc

    global BF, F32
    BF = mybir.dt.bfloat16
    F32 = mybir.dt.float32
    AF = mybir.ActivationFunctionType
    OP = mybir.AluOpType

    nc = bacc.Bacc(None, target_bir_lowering=False, debug=False)

    # ---- I/O ----
    xin_io = nc.dram_tensor("xin", [128, 6, S], BF, kind="ExternalInput")
    xq_io = nc.dram_tensor("xq", [128, 2, R], BF, kind="ExternalInput")
    xo_io = nc.dram_tensor("xo", [128, 6, R], F32, kind="ExternalInput")
    encq_io = nc.dram_tensor("encq", [128, 3, R], BF, kind="ExternalInput")
    enck_io = nc.dram_tensor("enck", [128, 3, S], BF, kind="ExternalInput")
    w_io = {}
    for nm, nin in [("wq", 2), ("wk", 2), ("wv", 2), ("wq2", 3), ("wk2", 3),
                    ("wv2", 6), ("wo2", 6), ("wf", 6)]:
        w_io[nm] = nc.dram_tensor(nm, [128, nin, D], BF, kind="ExternalInput")
    cvec_io = nc.dram_tensor("cvec", [128, 60], F32, kind="ExternalInput")
    brow_io = nc.dram_tensor("brow", [1, 2 * D], F32, kind="ExternalInput")
    sel_io = nc.dram_tensor("sel", [2, 128], BF, kind="ExternalInput")
    out_io = nc.dram_tensor("out", [128, 6, R], F32, kind="ExternalOutput")
    tap_io = {}
    if taps:
        for nm, shp, dt_ in [
            ("tq2", [128, 6, R], "bf"), ("tk2", [128, 6, S], "bf"),
            ("tqt", [128, 6, R], "bf"), ("tkt", [128, 6, S], "bf"),
            ("tvt", [128, 8, 780], "bf"), ("te0", [128, 1024], "bf"),
            ("tatt", [128, 6, R], "bf"), ("tden", [12, R], "bf"),
            ("tx1", [128, 6, R], "f"), ("tt", [128, 6, R], "f"),
            ("tv2f", [128, 8, 780], "bf"), ("tatt2", [128, 6, R], "bf"),
            ("tm2", [128, 6, R], "bf"),
            ("tx2", [128, 6, R], "f"), ("tt2", [128, 6, R], "f"),
            ("tout1", [128, 6, R], "f"),
        ]:
            tap_io[nm] = nc.dram_tensor(nm, shp, BF if dt_ == "bf" else F32,
                                        kind="ExternalOutput")

    PAIRS = [[0, 1], [2, 3], [4, 5], [6, 7]]
    ALL8 = [list(range(NC))]

    with tile.TileContext(nc) as tc:
        with (
            tc.tile_pool(name="pp", bufs=1) as pp,
            tc.tile_pool(name="trans", bufs=1) as tr,
            tc.tile_pool(name="resp", bufs=2) as resp,
            tc.tile_pool(name="attp", bufs=1) as attp,
            tc.tile_pool(name="expp", bufs=14) as expp,
            tc.tile_pool(name="smallp", bufs=1) as smallp,
            tc.tile_pool(name="ps", bufs=1, space="PSUM") as psp,
            tc.tile_pool(name="dram", bufs=1, space="DRAM") as dram,
        ):
            _psn = [0]

            def ps_s():
                # scores staging: [128, 1024] f32 = 2 banks, double buffered
                _psn[0] += 1
                return psp.tile([128, 1024], F32, tag="s", bufs=2,
                                name=f"ps_s{_psn[0]}")

            def ps_w():
                # work psum: AV accumulators / dense outputs / broadcasts
                _psn[0] += 1
                return psp.tile([128, 512], F32, tag="w", bufs=4,
                                name=f"ps_w{_psn[0]}")

            # ---- persistent SBUF ----
            cvec = pp.tile([128, 60], F32, name="sb_cvec")
            nc.sync.dma_start(cvec[:], cvec_io[:])
            # preamble-critical tensors first (parked in layer-scratch tags)
            encq_t = tr.tile([128, 6, R], BF, tag="qbf")
            encq = encq_t[:, 0:3, :]
            nc.sync.dma_start(encq, encq_io[:])
            enck_t = tr.tile([128, 6, S], BF, tag="kbf")
            enck = enck_t[:, 0:3, :]
            nc.sync.dma_start(enck, enck_io[:])
            w_sb = {}
            for nm in ("wq2", "wk2", "wq", "wk", "wv", "wv2", "wo2", "wf"):
                t_io = w_io[nm]
                w_sb[nm] = pp.tile(list(t_io.shape), BF, name=f"sb_{nm}")
                nc.sync.dma_start(w_sb[nm][:], t_io[:])
            xin = pp.tile([128, 6, S], BF, name="sb_xin")
            nc.sync.dma_start(xin[:], xin_io[:])
            xq1 = pp.tile([128, 2, R], BF, name="sb_xq1")
            nc.sync.dma_start(xq1[:], xq_io[:])
            xo = pp.tile([128, 6, R], F32, name="sb_xo")
            nc.sync.dma_start(xo[:], xo_io[:])

            zero_col = pp.tile([128, 1], F32, name="sb_zero")
            nc.vector.memset(zero_col[:], 0.0)
            eps_col = pp.tile([128, 1], F32, name="sb_eps")
            nc.vector.memset(eps_col[:], 1e-5)

            # selector for denominator broadcast: out[p] = rhs[p//64]
            sel = pp.tile([2, 128], BF, name="sb_sel")
            nc.sync.dma_start(sel[:], sel_io[:])

            # bias broadcast rows for token-major V / V2 evictions
            bias_bc = []
            for bi in range(2):
                t = pp.tile([128, D], F32, name=f"sb_biasbc{bi}")
                nc.sync.dma_start(
                    out=t[:, :],
                    in_=brow_io[0:1, bi * D:(bi + 1) * D].broadcast_to([128, D]))
                bias_bc.append(t)

            # cross-attn 1/denominators are layer-invariant; filled in layer 0
            dent2 = pp.tile([12, R], BF, name="sb_dent2")

            # ---- helpers ----
            def dense_fm(w, nin, rhs_fn, ncols, evict_fn):
                """out^T[128j+p, col] accumulation over nin input chunks."""
                for j in range(6):
                    for c0 in range(0, ncols, 512):
                        cw = min(512, ncols - c0)
                        ps = ps_w()
                        for i in range(nin):
                            nc.tensor.matmul(
                                ps[:, 0:cw],
                                w[:, i, j * 128:(j + 1) * 128],
                                rhs_fn(i, c0, cw),
                                start=(i == 0), stop=(i == nin - 1))
                        evict_fn(j, c0, cw, ps)

            def relu_evict(dst, base):
                """DVE eviction: relu(psum + bias_col)."""
                def f(j, c0, cw, ps):
                    nc.vector.tensor_scalar(
                        dst[:, j, c0:c0 + cw], ps[:, 0:cw],
                        cvec[:, base + j:base + j + 1], 0.0,
                        op0=OP.add, op1=OP.max)
                return f

            def iden_evict(dst, base):
                """DVE eviction: psum + bias_col."""
                def f(j, c0, cw, ps):
                    nc.vector.tensor_scalar_add(
                        dst[:, j, c0:c0 + cw], ps[:, 0:cw],
                        cvec[:, base + j:base + j + 1])
                return f

            def tokenmajor_vaug(w, bias_bc_t, x_lhs_fn, ntok, dst, relu):
                """V / V2 production: [tok, 12*65] with aug ones columns."""
                ntch = ntok // 128
                for tch in range(ntch):
                    nc.vector.memset(
                        dst[:, tch, :].rearrange("p (h k) -> p h k", k=65)[:, :, 64:65],
                        1.0)
                    for half in range(2):
                        ps = ps_w()
                        nin = w.shape[1]
                        for i in range(nin):
                            nc.tensor.matmul(
                                ps[:, 0:384],
                                x_lhs_fn(i, tch),
                                w[:, i, half * 384:(half + 1) * 384],
                                start=(i == 0), stop=(i == nin - 1))
                        nc.vector.tensor_tensor(
                            ps[:, 0:384], ps[:, 0:384],
                            bias_bc_t[:, half * 384:(half + 1) * 384], op=OP.add)
                        src = ps[:, 0:384].rearrange("p (h k) -> p h k", k=64)
                        dstap = dst[:, tch, :].rearrange(
                            "p (h k) -> p h k", k=65)[:, half * 6:(half + 1) * 6, 0:64]
                        if relu:
                            nc.scalar.activation(dstap, src, AF.Relu, bias=zero_col[:])
                        else:
                            nc.scalar.copy(dstap, src)

            def scores_exp(q_t, k_t, scale, p, jp, etap=None):
                """Head-pair p, key-chunk-pair jp -> (eA, eB) [128,1024] bf16.

                Per head: columns [chunk j, chunk j+1] with j = 2*jp.
                A = head 2p (features in partitions 0:64), B = head 2p+1.
                Scores run row-tiled: A on PE rows 0:64, B on rows 64:128,
                concurrently, into separate PSUM bank pairs.
                """
                sab = [ps_s(), ps_s()]
                j0 = 2 * jp
                for dj in range(2):
                    j = j0 + dj
                    for half in range(2):
                        off = 64 * half
                        nc.tensor.matmul(
                            sab[half][:, dj * 512:(dj + 1) * 512],
                            k_t[off:off + 64, p, j * 128:(j + 1) * 128],
                            q_t[off:off + 64, p, :],
                            start=True, stop=True)
                eab = []
                for half in range(2):
                    e = expp.tile([128, 1024], BF, tag="e",
                                  name=f"e{_psn[0]}_{half}")
                    nc.scalar.activation(e[:], sab[half][:], AF.Exp,
                                         bias=zero_col[:], scale=scale)
                    eab.append(e)
                if etap is not None:
                    nc.sync.dma_start(etap[:], eab[0][:])
                return eab

            _rrn = [0]

            def rr_tile():
                _rrn[0] += 1
                return smallp.tile([2, R], BF, tag="rrep", bufs=2,
                                   name=f"rr{_rrn[0]}")

            def recip_pair(dflat_t, dent2_dst):
                """dflat [1, 2R] (den A | den B) -> rrd [2, R] = 1/den."""
                rrd = rr_tile()
                nc.sync.dma_start(out=rrd[0:1, :], in_=dflat_t[0:1, 0:R])
                nc.sync.dma_start(out=rrd[1:2, :], in_=dflat_t[0:1, R:2 * R])
                with nc.allow_low_precision(reason="softmax denom recip, bf16"):
                    nc.vector.reciprocal(rrd[:], rrd[:])
                if dent2_dst is not None:
                    nc.sync.dma_start(out=dent2_dst, in_=rrd[:])
                return rrd

            def bcast_pair(rrd):
                """Broadcast rrd [2, R] across partitions: out[p] = rrd[p//64]."""
                ps = ps_w()
                nc.tensor.matmul(ps[:, 0:R], sel[:, :], rrd[:],
                                 start=True, stop=True)
                return ps


            _rrn = [0]

            def rr_tile():
                _rrn[0] += 1
                return smallp.tile([2, R], BF, tag="rrep", bufs=2,
                                   name=f"rr{_rrn[0]}")

            def recip_pair(dflat_t, dent2_dst):
                """dflat [1, 2R] (den A | den B) -> rrd [2, R] = 1/den."""
                rrd = rr_tile()
                nc.sync.dma_start(out=rrd[0:1, :], in_=dflat_t[0:1, 0:R])
                nc.sync.dma_start(out=rrd[1:2, :], in_=dflat_t[0:1, R:2 * R])
                with nc.allow_low_precision(reason="softmax denom recip, bf16"):
                    nc.vector.reciprocal(rrd[:], rrd[:])
                if dent2_dst is not None:
                    nc.sync.dma_start(out=dent2_dst, in_=rrd[:])
                return rrd

            def bcast_pair(rrd):
                """Broadcast rrd [2, R] across partitions: out[p] = rrd[p//64]."""
                ps = ps_w()
                nc.tensor.matmul(ps[:, 0:R], sel[:, :], rrd[:],
                                 start=True, stop=True)
                return ps

            bn_idx = [0]

            def bn_launch(stats):
                """DMA stats to DRAM + AllReduce launch. Returns arout."""
                i = bn_idx[0]
                bn_idx[0] += 1
                arin = dram.tile([128, 12], F32, tag=f"arin{i}")
                arout = dram.tile([128, 12], F32, tag=f"arout{i}",
                                  addr_space="Shared")
                nc.sync.dma_start(arin[:], stats[:])
                nc.gpsimd.collective_compute(
                    "AllReduce", OP.add, replica_groups=ALL8,
                    ins=[arin[:].opt()], outs=[arout[:].opt()])
                return arout

            def bn_start(res):
                """Stats of res [128,6,R] + AllReduce launch. Returns arout."""
                stats = smallp.tile([128, 12], F32, tag=f"stats{bn_idx[0]}",
                                    name=f"stats{bn_idx[0]}")
                for jh in range(6):
                    nc.vector.reduce_sum(stats[:, jh:jh + 1], res[:, jh, :],
                                         axis=mybir.AxisListType.X)
                for jh in range(6):
                    sq = ps_w()
                    nc.scalar.activation(sq[:, 0:R], res[:, jh, :], AF.Square,
                                         bias=zero_col[:],
                                         accum_out=stats[:, 6 + jh:7 + jh])
                return bn_launch(stats)

            def bn_finish(arout, gbase, bbase):
                """AR result -> per-feature scale (w[:,0:6]) / shift (w[:,6:12])."""
                i = bn_idx[0] - 1
                g = smallp.tile([128, 12], F32, tag=f"g{i}")
                nc.sync.dma_start(g[:], arout[:])
                w = smallp.tile([128, 18], F32, tag=f"bnw{i}")
                # mu = sum/N ; msq = sumsq/N
                nc.vector.tensor_scalar_mul(w[:, 12:18], g[:, 0:6], INV_N)
                nc.vector.tensor_scalar_mul(w[:, 6:12], g[:, 6:12], INV_N)
                # var = msq - mu^2
                nc.vector.tensor_tensor(w[:, 0:6], w[:, 12:18], w[:, 12:18],
                                        op=OP.mult)
                nc.vector.tensor_tensor(w[:, 0:6], w[:, 6:12], w[:, 0:6],
                                        op=OP.subtract)
                # std = sqrt(var + eps); rstd = 1/std
                nc.scalar.activation(w[:, 6:12], w[:, 0:6], AF.Sqrt, bias=eps_col[:])
                nc.vector.reciprocal(w[:, 0:6], w[:, 6:12])
                # scale = rstd*gamma ; shift = beta - mu*scale
                nc.vector.tensor_tensor(w[:, 0:6], w[:, 0:6],
                                        cvec[:, gbase:gbase + 6], op=OP.mult)
                nc.vector.tensor_tensor(w[:, 6:12], w[:, 12:18], w[:, 0:6],
                                        op=OP.mult)
                nc.vector.tensor_tensor(w[:, 6:12], cvec[:, bbase:bbase + 6],
                                        w[:, 6:12], op=OP.subtract)
                return w

            def bn_apply(w, src, dst, jh):
                """dst[:, jh, :] = src[:, jh, :] * scale + shift."""
                nc.vector.tensor_scalar(dst[:, jh, :], src[:, jh, :],
                                        w[:, 0 + jh:1 + jh], w[:, 6 + jh:7 + jh],
                                        op0=OP.mult, op1=OP.add)

            # ---- preamble: cross-attn Q2 (own rows), K2 (all rows) ----
            # computed once into DRAM; reloaded into qt/kt scratch per layer
            q2d = dram.tile([128, 6, R], BF, tag="q2d")
            k2d = dram.tile([128, 6, S], BF, tag="k2d")
            q2tmp = attp.tile([128, 6, R], BF, tag="att", name="q2tmp")
            dense_fm(w_sb["wq2"], 3, lambda i, c0, cw: encq[:, i, c0:c0 + cw], R,
                     iden_evict(q2tmp, 12))
            nc.sync.dma_start(q2d[:], q2tmp[:])
            for kh in range(2):
                k2tmp = attp.tile([128, 6, R], BF, tag="att",
                                  name=f"k2tmp{kh}")
                dense_fm(w_sb["wk2"], 3,
                         lambda i, c0, cw, kh=kh:
                         enck[:, i, kh * R + c0:kh * R + c0 + cw], R,
                         iden_evict(k2tmp, 18))
                nc.sync.dma_start(k2d[:, :, kh * R:(kh + 1) * R], k2tmp[:])
            if taps:
                nc.sync.dma_start(tap_io["tq2"][:], q2d[:])
                nc.sync.dma_start(tap_io["tk2"][:], k2d[:])

            # ---- layers ----
            xq_cur = xq1
            res_final = None
            for layer in range(NLAYERS):
                # Q (own rows), K (all rows) feature-major, with relu
                qt = tr.tile([128, 6, R], BF, tag="qbf")
                kt = tr.tile([128, 6, S], BF, tag="kbf")
                dense_fm(w_sb["wq"], 2,
                         lambda i, c0, cw: xq_cur[:, i, c0:c0 + cw], R,
                         relu_evict(qt, 0))
                dense_fm(w_sb["wk"], 2,
                         lambda i, c0, cw: xin[:, 2 + i, c0:c0 + cw], S,
                         relu_evict(kt, 6))
                # V token-major (all rows), relu, aug
                vt = tr.tile([128, 8, 780], BF, tag="vbf")
                tokenmajor_vaug(w_sb["wv"], bias_bc[0],
                                lambda i, tch: xin[:, 4 + i, tch * 128:(tch + 1) * 128],
                                S, vt, relu=True)
                if taps and layer == 0:
                    nc.sync.dma_start(tap_io["tqt"][:], qt[:])
                    nc.sync.dma_start(tap_io["tkt"][:], kt[:])
                    nc.sync.dma_start(tap_io["tvt"][:], vt[:])

                # ---- self attention (row-tiled scores, plain AV) ----
                att = attp.tile([128, 6, R], BF, tag="att")
                res = resp.tile([128, 6, R], F32, tag="res")
                stats = smallp.tile([128, 12], F32, tag=f"stats_a{layer}")
                for p in range(NPAIR):
                    dflat = smallp.tile([1, 2 * R], BF, tag="dflat", bufs=2,
                                        name=f"dflat{layer}_{p}")
                    po = [ps_w(), ps_w()]
                    for jp in range(4):
                        eab = scores_exp(
                            qt, kt, SCALE1, p, jp,
                            etap=tap_io.get("te0") if taps and layer == 0
                            and p == 0 and jp == 0 else None)
                        j0 = 2 * jp
                        for half in range(2):          # head A / head B
                            h = 2 * p + half
                            for dj in range(2):        # chunk j0+dj
                                j = j0 + dj
                                nc.tensor.matmul(
                                    po[half][0:65, 0:R],
                                    vt[:, j, h * 65:h * 65 + 65],
                                    eab[half][:, dj * 512:(dj + 1) * 512],
                                    start=(jp == 0 and dj == 0),
                                    stop=(jp == 3 and dj == 1))
                    for half in range(2):
                        h = 2 * p + half
                        off = 64 * half
                        nc.vector.tensor_copy(att[off:off + 64, p, :],
                                              po[half][0:64, 0:R])
                        nc.vector.tensor_copy(dflat[0:1, half * R:(half + 1) * R],
                                              po[half][64:65, 0:R])
                    # per-pair: 1/den -> broadcast -> x1 = att/den + xo -> stats
                    rrd = recip_pair(dflat, None)
                    bps = bcast_pair(rrd)
                    nc.vector.tensor_tensor(res[:, p, :], att[:, p, :],
                                            bps[:, 0:R], op=OP.mult)
                    nc.vector.tensor_tensor(res[:, p, :], res[:, p, :],
                                            xo[:, p, :], op=OP.add)
                    nc.vector.reduce_sum(stats[:, p:p + 1], res[:, p, :],
                                         axis=mybir.AxisListType.X)
                    sqp = ps_w()
                    nc.scalar.activation(sqp[:, 0:R], res[:, p, :], AF.Square,
                                         bias=zero_col[:],
                                         accum_out=stats[:, 6 + p:7 + p])
                if taps and layer == 0:
                    nc.sync.dma_start(tap_io["tatt"][:], att[:])
                    nc.sync.dma_start(tap_io["tx1"][:], res[:])

                # BN1 AllReduce, covered by cross-attn scores below
                ar1 = bn_launch(stats)

                # cross-attn scores+exp (independent of decoder state).
                # q2/k2 reload into the freed qt/kt scratch.
                q2t = tr.tile([128, 6, R], BF, tag="qbf", name=f"q2t{layer}")
                nc.sync.dma_start(q2t[:], q2d[:])
                k2t = tr.tile([128, 6, S], BF, tag="kbf", name=f"k2t{layer}")
                nc.sync.dma_start(k2t[:], k2d[:])
                e2s = []
                for p in range(NPAIR):
                    e2s.append([scores_exp(q2t, k2t, SCALE2, p, jp)
                                for jp in range(4)])

                w1 = bn_finish(ar1, 36, 42)  # g1, b1
                tbf = tr.tile([128, 6, R], BF, tag="tbf")
                for jh in range(6):
                    bn_apply(w1, res, tbf, jh)   # t in bf16 (for V2 proj)
                for jh in range(6):
                    bn_apply(w1, res, res, jh)   # t in f32 (residual)
                if taps and layer == 0:
                    nc.sync.dma_start(tap_io["tt"][:], res[:])

                # V2 own rows from t, then pair AllGather
                v2own = tr.tile([128, 4, 780], BF, tag="v2own")
                tokenmajor_vaug(w_sb["wv2"], bias_bc[1],
                                lambda i, tch: tbf[:, i, tch * 128:(tch + 1) * 128],
                                R, v2own, relu=False)
                agin = dram.tile([128, 4, 780], BF, tag=f"agin{layer}")
                agout = dram.tile([2, 128, 4, 780], BF, tag=f"agout{layer}")
                nc.sync.dma_start(agin[:], v2own[:])
                nc.gpsimd.collective_compute(
                    "AllGather", OP.bypass, replica_groups=PAIRS,
                    ins=[agin[:].opt()], outs=[agout[:].opt()])
                v2 = tr.tile([128, 8, 780], BF, tag="v2full")
                nc.sync.dma_start(v2[:, 0:4, :], agout[0, :, :, :])
                nc.sync.dma_start(v2[:, 4:8, :], agout[1, :, :, :])
                if taps and layer == 0:
                    nc.sync.dma_start(tap_io["tv2f"][:], v2[:])

                # cross AV (full-array, accumulating all 8 chunks per head)
                att2 = attp.tile([128, 6, R], BF, tag="att")
                m2 = tr.tile([128, 6, R], BF, tag="tbf")
                for p in range(NPAIR):
                    dflat2 = None
                    if layer == 0:
                        dflat2 = smallp.tile([1, 2 * R], BF, tag="dflat",
                                             bufs=2, name=f"dflat2_{p}")
                    for half in range(2):
                        h = 2 * p + half
                        off = 64 * half
                        po = ps_w()
                        for j in range(NKCH):
                            e = e2s[p][j // 2][half]
                            blk = (j % 2) * 512
                            nc.tensor.matmul(
                                po[0:65, 0:R],
                                v2[:, j, h * 65:h * 65 + 65],
                                e[:, blk:blk + 512],
                                start=(j == 0), stop=(j == NKCH - 1))
                        nc.vector.tensor_copy(att2[off:off + 64, p, :],
                                              po[0:64, 0:R])
                        if layer == 0:
                            nc.vector.tensor_copy(
                                dflat2[0:1, half * R:(half + 1) * R],
                                po[64:65, 0:R])
                    if layer == 0:
                        rrd = recip_pair(dflat2, dent2[2 * p:2 * p + 2, :])
                    else:
                        rrd = rr_tile()
                        nc.sync.dma_start(out=rrd[:],
                                          in_=dent2[2 * p:2 * p + 2, :])
                    bps = bcast_pair(rrd)
                    nc.vector.tensor_tensor(m2[:, p, :], att2[:, p, :],
                                            bps[:, 0:R], op=OP.mult)
                if taps and layer == 0:
                    nc.sync.dma_start(tap_io["tatt2"][:], att2[:])
                    nc.sync.dma_start(tap_io["tm2"][:], m2[:])

                # x2 = m2 @ Wo2 + bo2 + t
                res2 = resp.tile([128, 6, R], F32, tag="res")
                t_prev = res
                dense_fm(w_sb["wo2"], 6,
                         lambda i, c0, cw: m2[:, i, c0:c0 + cw], R,
                         lambda j, c0, cw, ps: nc.vector.scalar_tensor_tensor(
                             res2[:, j, c0:c0 + cw], ps[:, 0:cw],
                             cvec[:, 24 + j:25 + j], t_prev[:, j, c0:c0 + cw],
                             op0=OP.add, op1=OP.add))
                if taps and layer == 0:
                    nc.sync.dma_start(tap_io["tx2"][:], res2[:])
                ar2 = bn_start(res2)
                w2 = bn_finish(ar2, 48, 54)  # g2, b2
                t2bf = tr.tile([128, 6, R], BF, tag="tbf")
                for jh in range(6):
                    bn_apply(w2, res2, t2bf, jh)
                for jh in range(6):
                    bn_apply(w2, res2, res2, jh)
                if taps and layer == 0:
                    nc.sync.dma_start(tap_io["tt2"][:], res2[:])

                # FFN: x3 = t2 @ Wf + bf + t2
                res3 = resp.tile([128, 6, R], F32, tag="res")
                dense_fm(w_sb["wf"], 6,
                         lambda i, c0, cw: t2bf[:, i, c0:c0 + cw], R,
                         lambda j, c0, cw, ps: nc.vector.scalar_tensor_tensor(
                             res3[:, j, c0:c0 + cw], ps[:, 0:cw],
                             cvec[:, 30 + j:31 + j], res2[:, j, c0:c0 + cw],
                             op0=OP.add, op1=OP.add))

                if layer < NLAYERS - 1:
                    # AllGather pre-BN x3 (bf16) concurrently with stats AR
                    xcast = tr.tile([128, 6, R], BF, tag="xcast")
                    for jh in range(6):
                        nc.vector.tensor_copy(xcast[:, jh, :], res3[:, jh, :])
                    xagin = dram.tile([128, 6, R], BF, tag="xagin")
                    xagout = dram.tile([2, 128, 6, R], BF, tag="xagout")
                    nc.sync.dma_start(xagin[:], xcast[:])
                    nc.gpsimd.collective_compute(
                        "AllGather", OP.bypass, replica_groups=PAIRS,
                        ins=[xagin[:].opt()], outs=[xagout[:].opt()])
                    ar3 = bn_start(res3)
                    nc.sync.dma_start(xin[:, :, 0:R], xagout[0, :, :, :])
                    nc.sync.dma_start(xin[:, :, R:S], xagout[1, :, :, :])
                    w3 = bn_finish(ar3, 48, 54)
                    # next-layer inputs: xin (all rows bf16), xq (own bf16),
                    # xo (own f32) -- all post-BN
                    xnew = tr.tile([128, 6, R], BF, tag="xcast")
                    for jh in range(6):
                        bn_apply(w3, res3, xnew, jh)
                        nc.vector.tensor_scalar(
                            xo[:, jh, :], res3[:, jh, :],
                            w3[:, 0 + jh:1 + jh], w3[:, 6 + jh:7 + jh],
                            op0=OP.mult, op1=OP.add)
                    for jh in range(6):
                        nc.vector.tensor_scalar(
                            xin[:, jh, :], xin[:, jh, :],
                            w3[:, 0 + jh:1 + jh], w3[:, 6 + jh:7 + jh],
                            op0=OP.mult, op1=OP.add)
                    if taps and layer == 0:
                        nc.sync.dma_start(tap_io["tout1"][:], xo[:])
                    xq_cur = xnew[:, 0:2, :]
                else:
                    ar3 = bn_start(res3)
                    w3 = bn_finish(ar3, 48, 54)
                    for jh in range(6):
                        bn_apply(w3, res3, res3, jh)
                    res_final = res3

            nc.sync.dma_start(out_io[:], res_final[:])

    nc.compile()
    return nc


def _host_prepare(inputs):
    x = np.asarray(inputs["x"])
    encod = np.asarray(inputs["encod"], np.float32)
    embed = np.asarray(inputs["embed"], np.float32)
    emb = embed[x.astype(np.int64)]
    im0 = 2.0 * emb + _pos_encoding()[None]  # [B,S,D] f32

    wq, wk, wv = (np.asarray(inputs[k], np.float32) for k in ("Wq", "Wk", "Wv"))
    wq2, wk2 = (np.asarray(inputs[k], np.float32) for k in ("Wq2", "Wk2"))
    wv2, wo2, wf = (np.asarray(inputs[k], np.float32) for k in ("Wv2", "Wo2", "Wf"))
    w_np = {nm: _bf16(_wchunk(w)) for nm, w in
            [("wq", wq), ("wk", wk), ("wv", wv), ("wq2", wq2), ("wk2", wk2),
             ("wv2", wv2), ("wo2", wo2), ("wf", wf)]}
    cvec = np.concatenate(
        [_col(np.asarray(inputs[k], np.float32)) for k in
         ("bq", "bk", "bq2", "bk2", "bo2", "bf", "g1", "b1", "g2", "b2")],
        axis=1).astype(np.float32)
    brow = np.concatenate([np.asarray(inputs["bv"], np.float32),
                           np.asarray(inputs["bv2"], np.float32)])[None, :].astype(np.float32)
    sel = np.zeros((2, 128), np.float32)
    sel[0, 0:64] = 1.0
    sel[1, 64:128] = 1.0

    in_maps = []
    for c in range(NC):
        b_, r_ = c // 2, c % 2
        rows = slice(r_ * R, (r_ + 1) * R)
        m = dict(w_np)
        m["cvec"] = cvec
        m["brow"] = brow
        m["sel"] = _bf16(sel)
        m["xin"] = _bf16(_fm(im0[b_]))
        m["xq"] = _bf16(_fm(im0[b_][rows, 0:256]))
        m["xo"] = _fm(im0[b_][rows]).astype(np.float32)
        m["encq"] = _bf16(_fm(encod[b_][rows, 0:384]))
        m["enck"] = _bf16(_fm(encod[b_][:, 384:768]))
        in_maps.append(m)
    return in_maps


def _gather(results):
    out = np.zeros((B, S, D), np.float32)
    for c in range(NC):
        b_, r_ = c // 2, c % 2
        a = results[c]["out"]  # [128, 6, R]
        out[b_, r_ * R:(r_ + 1) * R] = a.transpose(1, 0, 2).reshape(D, R).T
    return out


def kernel(**inputs) -> np.ndarray:
    from concourse.bass_utils import run_bass_kernel_spmd

    if "nc" not in _CACHE:
        _CACHE["nc"] = _build()
    nc = _CACHE["nc"]
    in_maps = _host_prepare(inputs)
    res = run_bass_kernel_spmd(nc, in_maps, core_ids=list(range(NC)))
    return _gather(res.results)


# revision 29
# speedup vs baseline: 1.3359x; 1.3359x over previous
"""Trainium2 Bass kernel for nn_Decoder (dense transformer decoder, 2 layers).

Sharding (8 cores): core c = 2*b + r handles batch b, query-row half r.
- Attention (scores/softmax/AV, all heads) is split by query rows.
- K/V projections are computed for all rows (duplicated within the pair).
- Cross-attention V2 is computed for own rows then pair-AllGathered.
- BatchNorm statistics are 8-rank AllReduced (sums over all B*S rows).
- Layer boundary: pair-AllGather of the new input_multi halves.

v2 rewrite vs baseline:
- Scores matmuls are 64-contraction row-tiled (two heads run concurrently on
  PE half-arrays, uniform (64,128) tile mode within self-attention).
- Scores land in PSUM as bf16: one [128,2048] bank-pair holds a head-pair x
  2 key-chunks, consumed by a single EXP instruction.
- AV is row-tiled over kpos halves into two accumulator banks (P/Q) with the
  V-aug ones column producing denominators in row 64.
- Softmax denominators: DVE adds into a [12,512] tile, one batched
  reciprocal, a rearrange-DMA into [2, 6*512], then one tiny selector-matmul
  per head pair broadcasts 1/den across 128 partitions in PSUM.
- Cross-attention scores+exp depend only on encod, so they are issued to
  cover the BN1 AllReduce and the V2 AllGather. Cross denominators are
  layer-invariant and cached from layer 0.
- Layer boundary: AllGather of the pre-BN FFN residual runs concurrently
  with the BN-stats AllReduce; the affine is applied locally afterwards.

Layout: activations are feature-major ("X^T", [feat, token]) stored as
[128, chunk, tok] SBUF tiles (feature f = 128*chunk + partition).
V / V2 are token-major [tok, head*65] with a ones column appended per head
(V-aug) so softmax denominators fall out of the AV matmul as row 64.
All matmuls are bf16 x bf16; the residual stream and BN statistics are f32.
"""
import numpy as np
import ml_dtypes

B, S, D, H, VOCAB, NLAYERS = 4, 1024, 768, 12, 32000, 2
HD = D // H          # 64
R = S // 2           # 512 own rows per core
NC = 8
SCALE1 = 1.0 / float(np.sqrt(D))
SCALE2 = 1.0 / float(np.sqrt(HD))
INV_N = 1.0 / (B * S)
NKCH = S // 128      # 8 key chunks
NPAIR = H // 2       # 6 head pairs

BF = None
F32 = None

_CACHE = {}


def _pos_encoding():
    p = np.arange(S, dtype=np.float32)[:, None]
    i = np.arange(D // 2, dtype=np.float32)[None, :]
    ang = p / np.power(10000.0, 2.0 * i / D)
    return np.stack([np.sin(ang), np.cos(ang)], axis=-1).reshape(S, D).astype(np.float32)


def _fm(a):
    """[tok, feat] -> feature-major chunked [128, nchunk, tok]."""
    t, f = a.shape
    return np.ascontiguousarray(a.T.reshape(f // 128, 128, t).transpose(1, 0, 2))


def _wchunk(w):
    """[in, out] weight -> [128, nin, out] (stationary chunks)."""
    i, o = w.shape
    return np.ascontiguousarray(w.reshape(i // 128, 128, o).transpose(1, 0, 2))


def _col(v):
    """[768] -> [128, 6] feature-major columns."""
    return np.ascontiguousarray(v.reshape(6, 128).T)


def _bf16(a):
    return np.asarray(a, np.float32).astype(ml_dtypes.bfloat16)


def _build(taps=False):
    import concourse.bass as bass
    import concourse.mybir as mybir
    import concourse.tile as tile
    from concourse import bacc

    global BF, F32
    BF = mybir.dt.bfloat16
    F32 = mybir.dt.float32
    AF = mybir.ActivationFunctionType
    OP = mybir.AluOpType

    nc = bacc.Bacc(None, target_bir_lowering=False, debug=False)

    # ---- I/O ----
    xin_io = nc.dram_tensor("xin", [128, 6, S], BF, kind="ExternalInput")
    xq_io = nc.dram_tensor("xq", [128, 2, R], BF, kind="ExternalInput")
    xo_io = nc.dram_tensor("xo", [128, 6, R], F32, kind="ExternalInput")
    encq_io = nc.dram_tensor("encq", [128, 3, R], BF, kind="ExternalInput")
    enck_io = nc.dram_tensor("enck", [128, 3, S], BF, kind="ExternalInput")
    w_io = {}
    for nm, nin in [("wq", 2), ("wk", 2), ("wv", 2), ("wq2", 3), ("wk2", 3),
                    ("wv2", 6), ("wo2", 6), ("wf", 6)]:
        w_io[nm] = nc.dram_tensor(nm, [128, nin, D], BF, kind="ExternalInput")
    cvec_io = nc.dram_tensor("cvec", [128, 60], F32, kind="ExternalInput")
    brow_io = nc.dram_tensor("brow", [1, 2 * D], F32, kind="ExternalInput")
    sel_io = nc.dram_tensor("sel", [2, 128], BF, kind="ExternalInput")
    out_io = nc.dram_tensor("out", [128, 6, R], F32, kind="ExternalOutput")
    tap_io = {}
    if taps:
        for nm, shp, dt_ in [
            ("tq2", [128, 6, R], "bf"), ("tk2", [128, 6, S], "bf"),
            ("tqt", [128, 6, R], "bf"), ("tkt", [128, 6, S], "bf"),
            ("tvt", [128, 8, 780], "bf"), ("te0", [128, 1024], "bf"),
            ("tatt", [128, 6, R], "bf"), ("tden", [12, R], "bf"),
            ("tx1", [128, 6, R], "f"), ("tt", [128, 6, R], "f"),
            ("tv2f", [128, 8, 780], "bf"), ("tatt2", [128, 6, R], "bf"),
            ("tm2", [128, 6, R], "bf"),
            ("tx2", [128, 6, R], "f"), ("tt2", [128, 6, R], "f"),
            ("tout1", [128, 6, R], "f"),
        ]:
            tap_io[nm] = nc.dram_tensor(nm, shp, BF if dt_ == "bf" else F32,
                                        kind="ExternalOutput")

    PAIRS = [[0, 1], [2, 3], [4, 5], [6, 7]]
    ALL8 = [list(range(NC))]

    with tile.TileContext(nc) as tc:
        with (
            tc.tile_pool(name="pp", bufs=1) as pp,
            tc.tile_pool(name="trans", bufs=1) as tr,
            tc.tile_pool(name="resp", bufs=2) as resp,
            tc.tile_pool(name="attp", bufs=1) as attp,
            tc.tile_pool(name="expp", bufs=13) as expp,
            tc.tile_pool(name="smallp", bufs=1) as smallp,
            tc.tile_pool(name="ps", bufs=1, space="PSUM") as psp,
            tc.tile_pool(name="dram", bufs=1, space="DRAM") as dram,
        ):
            _psn = [0]

            def ps_s():
                # scores staging: [128, 1024] f32 = 2 banks, double buffered
                _psn[0] += 1
                return psp.tile([128, 1024], F32, tag="s", bufs=2,
                                name=f"ps_s{_psn[0]}")

            def ps_w():
                # work psum: AV accumulators / dense outputs / broadcasts
                _psn[0] += 1
                return psp.tile([128, 512], F32, tag="w", bufs=4,
                                name=f"ps_w{_psn[0]}")

            # ---- persistent SBUF ----
            cvec = pp.tile([128, 60], F32, name="sb_cvec")
            nc.sync.dma_start(cvec[:], cvec_io[:])
            # preamble-critical tensors first (parked in layer-scratch tags)
            encq_t = tr.tile([128, 6, R], BF, tag="qbf")
            encq = encq_t[:, 0:3, :]
            nc.sync.dma_start(encq, encq_io[:])
            enck_t = tr.tile([128, 6, S], BF, tag="kbf")
            enck = enck_t[:, 0:3, :]
            nc.sync.dma_start(enck, enck_io[:])
            w_sb = {}
            for nm in ("wq2", "wk2", "wq", "wk", "wv", "wv2", "wo2", "wf"):
                t_io = w_io[nm]
                w_sb[nm] = pp.tile(list(t_io.shape), BF, name=f"sb_{nm}")
                nc.sync.dma_start(w_sb[nm][:], t_io[:])
            xin = pp.tile([128, 6, S], BF, name="sb_xin")
            nc.sync.dma_start(xin[:], xin_io[:])
            xq1 = pp.tile([128, 2, R], BF, name="sb_xq1")
            nc.sync.dma_start(xq1[:], xq_io[:])
            xo = pp.tile([128, 6, R], F32, name="sb_xo")
            nc.sync.dma_start(xo[:], xo_io[:])

            zero_col = pp.tile([128, 1], F32, name="sb_zero")
            nc.vector.memset(zero_col[:], 0.0)
            eps_col = pp.tile([128, 1], F32, name="sb_eps")
            nc.vector.memset(eps_col[:], 1e-5)

            # selector for denominator broadcast: out[p] = rhs[p//64]
            sel = pp.tile([2, 128], BF, name="sb_sel")
            nc.sync.dma_start(sel[:], sel_io[:])

            # bias broadcast rows for token-major V / V2 evictions
            bias_bc = []
            for bi in range(2):
                t = pp.tile([128, D], F32, name=f"sb_biasbc{bi}")
                nc.sync.dma_start(
                    out=t[:, :],
                    in_=brow_io[0:1, bi * D:(bi + 1) * D].broadcast_to([128, D]))
                bias_bc.append(t)

            # cross-attn 1/denominators are layer-invariant; filled in layer 0
            dent2 = pp.tile([12, R], BF, name="sb_dent2")

            # ---- helpers ----
            def dense_fm(w, nin, rhs_fn, ncols, evict_fn):
                """out^T[128j+p, col] accumulation over nin input chunks."""
                for j in range(6):
                    for c0 in range(0, ncols, 512):
                        cw = min(512, ncols - c0)
                        ps = ps_w()
                        for i in range(nin):
                            nc.tensor.matmul(
                                ps[:, 0:cw],
                                w[:, i, j * 128:(j + 1) * 128],
                                rhs_fn(i, c0, cw),
                                start=(i == 0), stop=(i == nin - 1))
                        evict_fn(j, c0, cw, ps)

            def relu_evict(dst, base):
                """DVE eviction: relu(psum + bias_col)."""
                def f(j, c0, cw, ps):
                    nc.vector.tensor_scalar(
                        dst[:, j, c0:c0 + cw], ps[:, 0:cw],
                        cvec[:, base + j:base + j + 1], 0.0,
                        op0=OP.add, op1=OP.max)
                return f

            def iden_evict(dst, base):
                """DVE eviction: psum + bias_col."""
                def f(j, c0, cw, ps):
                    nc.vector.tensor_scalar_add(
                        dst[:, j, c0:c0 + cw], ps[:, 0:cw],
                        cvec[:, base + j:base + j + 1])
                return f

            def tokenmajor_vaug(w, bias_bc_t, x_lhs_fn, ntok, dst, relu):
                """V / V2 production: [tok, 12*65] with aug ones columns."""
                ntch = ntok // 128
                for tch in range(ntch):
                    nc.vector.memset(
                        dst[:, tch, :].rearrange("p (h k) -> p h k", k=65)[:, :, 64:65],
                        1.0)
                    for half in range(2):
                        ps = ps_w()
                        nin = w.shape[1]
                        for i in range(nin):
                            nc.tensor.matmul(
                                ps[:, 0:384],
                                x_lhs_fn(i, tch),
                                w[:, i, half * 384:(half + 1) * 384],
                                start=(i == 0), stop=(i == nin - 1))
                        nc.vector.tensor_tensor(
                            ps[:, 0:384], ps[:, 0:384],
                            bias_bc_t[:, half * 384:(half + 1) * 384], op=OP.add)
                        src = ps[:, 0:384].rearrange("p (h k) -> p h k", k=64)
                        dstap = dst[:, tch, :].rearrange(
                            "p (h k) -> p h k", k=65)[:, half * 6:(half + 1) * 6, 0:64]
                        if relu:
                            nc.scalar.activation(dstap, src, AF.Relu, bias=zero_col[:])
                        else:
                            nc.scalar.copy(dstap, src)

            def scores_exp(q_t, k_t, scale, p, jp, etap=None):
                """Head-pair p, key-chunk-pair jp -> (eA, eB) [128,1024] bf16.

                Per head: columns [chunk j, chunk j+1] with j = 2*jp.
                A = head 2p (features in partitions 0:64), B = head 2p+1.
                Scores run row-tiled: A on PE rows 0:64, B on rows 64:128,
                concurrently, into separate PSUM bank pairs.
                """
                sab = [ps_s(), ps_s()]
                j0 = 2 * jp
                for dj in range(2):
                    j = j0 + dj
                    for half in range(2):
                        off = 64 * half
                        nc.tensor.matmul(
                            sab[half][:, dj * 512:(dj + 1) * 512],
                            k_t[off:off + 64, p, j * 128:(j + 1) * 128],
                            q_t[off:off + 64, p, :],
                            start=True, stop=True)
                eab = []
                for half in range(2):
                    e = expp.tile([128, 1024], BF, tag="e",
                                  name=f"e{_psn[0]}_{half}")
                    nc.scalar.activation(e[:], sab[half][:], AF.Exp,
                                         bias=zero_col[:], scale=scale)
                    eab.append(e)
                if etap is not None:
                    nc.sync.dma_start(etap[:], eab[0][:])
                return eab

            _rrn = [0]

            def rr_tile():
                _rrn[0] += 1
                return smallp.tile([2, R], BF, tag="rrep", bufs=6,
                                   name=f"rr{_rrn[0]}")

            def recip_pair(dflat_t, dent2_dst):
                """dflat [1, 2R] (den A | den B) -> rrd [2, R] = 1/den."""
                rrd = rr_tile()
                nc.sync.dma_start(out=rrd[0:1, :], in_=dflat_t[0:1, 0:R])
                nc.sync.dma_start(out=rrd[1:2, :], in_=dflat_t[0:1, R:2 * R])
                with nc.allow_low_precision(reason="softmax denom recip, bf16"):
                    nc.vector.reciprocal(rrd[:], rrd[:])
                if dent2_dst is not None:
                    nc.sync.dma_start(out=dent2_dst, in_=rrd[:])
                return rrd

            def bcast_pair(rrd):
                """Broadcast rrd [2, R] across partitions: out[p] = rrd[p//64]."""
                ps = ps_w()
                nc.tensor.matmul(ps[:, 0:R], sel[:, :], rrd[:],
                                 start=True, stop=True)
                return ps


            _rrn = [0]

            def rr_tile():
                _rrn[0] += 1
                return smallp.tile([2, R], BF, tag="rrep", bufs=6,
                                   name=f"rr{_rrn[0]}")

            def recip_pair(dflat_t, dent2_dst):
                """dflat [1, 2R] (den A | den B) -> rrd [2, R] = 1/den."""
                rrd = rr_tile()
                nc.sync.dma_start(out=rrd[0:1, :], in_=dflat_t[0:1, 0:R])
                nc.sync.dma_start(out=rrd[1:2, :], in_=dflat_t[0:1, R:2 * R])
                with nc.allow_low_precision(reason="softmax denom recip, bf16"):
                    nc.vector.reciprocal(rrd[:], rrd[:])
                if dent2_dst is not None:
                    nc.sync.dma_start(out=dent2_dst, in_=rrd[:])
                return rrd

            def bcast_pair(rrd):
                """Broadcast rrd [2, R] across partitions: out[p] = rrd[p//64]."""
                ps = ps_w()
                nc.tensor.matmul(ps[:, 0:R], sel[:, :], rrd[:],
                                 start=True, stop=True)
                return ps

            bn_idx = [0]

            def bn_launch(stats):
                """DMA stats to DRAM + AllReduce launch. Returns arout."""
                i = bn_idx[0]
                bn_idx[0] += 1
                arin = dram.tile([128, 12], F32, tag=f"arin{i}")
                arout = dram.tile([128, 12], F32, tag=f"arout{i}",
                                  addr_space="Shared")
                nc.sync.dma_start(arin[:], stats[:])
                nc.gpsimd.collective_compute(
                    "AllReduce", OP.add, replica_groups=ALL8,
                    ins=[arin[:].opt()], outs=[arout[:].opt()])
                return arout

            def bn_start(res):
                """Stats of res [128,6,R] + AllReduce launch. Returns arout."""
                stats = smallp.tile([128, 12], F32, tag=f"stats{bn_idx[0]}",
                                    name=f"stats{bn_idx[0]}")
                for jh in range(6):
                    nc.vector.reduce_sum(stats[:, jh:jh + 1], res[:, jh, :],
                                         axis=mybir.AxisListType.X)
                for jh in range(6):
                    sq = ps_w()
                    nc.scalar.activation(sq[:, 0:R], res[:, jh, :], AF.Square,
                                         bias=zero_col[:],
                                         accum_out=stats[:, 6 + jh:7 + jh])
                return bn_launch(stats)

            def bn_finish(arout, gbase, bbase):
                """AR result -> per-feature scale (w[:,0:6]) / shift (w[:,6:12])."""
                i = bn_idx[0] - 1
                g = smallp.tile([128, 12], F32, tag=f"g{i}")
                nc.sync.dma_start(g[:], arout[:])
                w = smallp.tile([128, 18], F32, tag=f"bnw{i}")
                # mu = sum/N ; msq = sumsq/N
                nc.vector.tensor_scalar_mul(w[:, 12:18], g[:, 0:6], INV_N)
                nc.vector.tensor_scalar_mul(w[:, 6:12], g[:, 6:12], INV_N)
                # var = msq - mu^2
                nc.vector.tensor_tensor(w[:, 0:6], w[:, 12:18], w[:, 12:18],
                                        op=OP.mult)
                nc.vector.tensor_tensor(w[:, 0:6], w[:, 6:12], w[:, 0:6],
                                        op=OP.subtract)
                # std = sqrt(var + eps); rstd = 1/std
                nc.scalar.activation(w[:, 6:12], w[:, 0:6], AF.Sqrt, bias=eps_col[:])
                nc.vector.reciprocal(w[:, 0:6], w[:, 6:12])
                # scale = rstd*gamma ; shift = beta - mu*scale
                nc.vector.tensor_tensor(w[:, 0:6], w[:, 0:6],
                                        cvec[:, gbase:gbase + 6], op=OP.mult)
                nc.vector.tensor_tensor(w[:, 6:12], w[:, 12:18], w[:, 0:6],
                                        op=OP.mult)
                nc.vector.tensor_tensor(w[:, 6:12], cvec[:, bbase:bbase + 6],
                                        w[:, 6:12], op=OP.subtract)
                return w

            def bn_apply(w, src, dst, jh):
                """dst[:, jh, :] = src[:, jh, :] * scale + shift."""
                nc.vector.tensor_scalar(dst[:, jh, :], src[:, jh, :],
                                        w[:, 0 + jh:1 + jh], w[:, 6 + jh:7 + jh],
                                        op0=OP.mult, op1=OP.add)

            # ---- preamble: cross-attn Q2 (own rows), K2 (all rows) ----
            # computed once into DRAM; reloaded into qt/kt scratch per layer
            q2d = dram.tile([128, 6, R], BF, tag="q2d")
            k2d = dram.tile([128, 6, S], BF, tag="k2d")
            q2tmp = attp.tile([128, 6, R], BF, tag="att", name="q2tmp")
            dense_fm(w_sb["wq2"], 3, lambda i, c0, cw: encq[:, i, c0:c0 + cw], R,
                     iden_evict(q2tmp, 12))
            nc.sync.dma_start(q2d[:], q2tmp[:])
            for kh in range(2):
                k2tmp = attp.tile([128, 6, R], BF, tag="att",
                                  name=f"k2tmp{kh}")
                dense_fm(w_sb["wk2"], 3,
                         lambda i, c0, cw, kh=kh:
                         enck[:, i, kh * R + c0:kh * R + c0 + cw], R,
                         iden_evict(k2tmp, 18))
                nc.sync.dma_start(k2d[:, :, kh * R:(kh + 1) * R], k2tmp[:])
            if taps:
                nc.sync.dma_start(tap_io["tq2"][:], q2d[:])
                nc.sync.dma_start(tap_io["tk2"][:], k2d[:])

            # ---- layers ----
            xq_cur = xq1
            res_final = None
            for layer in range(NLAYERS):
                # Q (own rows), K (all rows) feature-major, with relu
                qt = tr.tile([128, 6, R], BF, tag="qbf")
                kt = tr.tile([128, 6, S], BF, tag="kbf")
                dense_fm(w_sb["wq"], 2,
                         lambda i, c0, cw: xq_cur[:, i, c0:c0 + cw], R,
                         relu_evict(qt, 0))
                dense_fm(w_sb["wk"], 2,
                         lambda i, c0, cw: xin[:, 2 + i, c0:c0 + cw], S,
                         relu_evict(kt, 6))
                # V token-major (all rows), relu, aug
                vt = tr.tile([128, 8, 780], BF, tag="vbf")
                tokenmajor_vaug(w_sb["wv"], bias_bc[0],
                                lambda i, tch: xin[:, 4 + i, tch * 128:(tch + 1) * 128],
                                S, vt, relu=True)
                if taps and layer == 0:
                    nc.sync.dma_start(tap_io["tqt"][:], qt[:])
                    nc.sync.dma_start(tap_io["tkt"][:], kt[:])
                    nc.sync.dma_start(tap_io["tvt"][:], vt[:])

                # ---- self attention (row-tiled scores, plain AV) ----
                att = attp.tile([128, 6, R], BF, tag="att")
                res = resp.tile([128, 6, R], F32, tag="res")
                stats = smallp.tile([128, 12], F32, tag=f"stats_a{layer}")
                rrds = []
                for p in range(NPAIR):
                    dflat = smallp.tile([1, 2 * R], BF, tag="dflat", bufs=2,
                                        name=f"dflat{layer}_{p}")
                    po = [ps_w(), ps_w()]
                    for jp in range(4):
                        eab = scores_exp(
                            qt, kt, SCALE1, p, jp,
                            etap=tap_io.get("te0") if taps and layer == 0
                            and p == 0 and jp == 0 else None)
                        j0 = 2 * jp
                        for half in range(2):          # head A / head B
                            h = 2 * p + half
                            for dj in range(2):        # chunk j0+dj
                                j = j0 + dj
                                nc.tensor.matmul(
                                    po[half][0:65, 0:R],
                                    vt[:, j, h * 65:h * 65 + 65],
                                    eab[half][:, dj * 512:(dj + 1) * 512],
                                    start=(jp == 0 and dj == 0),
                                    stop=(jp == 3 and dj == 1))
                    for half in range(2):
                        h = 2 * p + half
                        off = 64 * half
                        nc.vector.tensor_copy(att[off:off + 64, p, :],
                                              po[half][0:64, 0:R])
                        nc.vector.tensor_copy(dflat[0:1, half * R:(half + 1) * R],
                                              po[half][64:65, 0:R])
                    rrds.append(recip_pair(dflat, None))
                # tail: broadcasts fire with all reciprocals ready (no PE stall)
                for p in range(NPAIR):
                    bps = bcast_pair(rrds[p])
                    nc.vector.tensor_tensor(res[:, p, :], att[:, p, :],
                                            bps[:, 0:R], op=OP.mult)
                    nc.vector.tensor_tensor(res[:, p, :], res[:, p, :],
                                            xo[:, p, :], op=OP.add)
                    nc.vector.reduce_sum(stats[:, p:p + 1], res[:, p, :],
                                         axis=mybir.AxisListType.X)
                    sqp = ps_w()
                    nc.scalar.activation(sqp[:, 0:R], res[:, p, :], AF.Square,
                                         bias=zero_col[:],
                                         accum_out=stats[:, 6 + p:7 + p])
                if taps and layer == 0:
                    nc.sync.dma_start(tap_io["tatt"][:], att[:])
                    nc.sync.dma_start(tap_io["tx1"][:], res[:])

                # BN1 AllReduce, covered by cross-attn scores below
                ar1 = bn_launch(stats)

                # cross-attn scores+exp (independent of decoder state).
                # q2/k2 reload into the freed qt/kt scratch.
                q2t = tr.tile([128, 6, R], BF, tag="qbf", name=f"q2t{layer}")
                nc.sync.dma_start(q2t[:], q2d[:])
                k2t = tr.tile([128, 6, S], BF, tag="kbf", name=f"k2t{layer}")
                nc.sync.dma_start(k2t[:], k2d[:])
                e2s = []
                for p in range(NPAIR):
                    e2s.append([scores_exp(q2t, k2t, SCALE2, p, jp)
                                for jp in range(4)])

                w1 = bn_finish(ar1, 36, 42)  # g1, b1
                tbf = tr.tile([128, 6, R], BF, tag="tbf")
                for jh in range(6):
                    bn_apply(w1, res, tbf, jh)   # t in bf16 (for V2 proj)
                for jh in range(6):
                    bn_apply(w1, res, res, jh)   # t in f32 (residual)
                if taps and layer == 0:
                    nc.sync.dma_start(tap_io["tt"][:], res[:])

                # V2 own rows from t, then pair AllGather
                v2own = tr.tile([128, 4, 780], BF, tag="v2own")
                tokenmajor_vaug(w_sb["wv2"], bias_bc[1],
                                lambda i, tch: tbf[:, i, tch * 128:(tch + 1) * 128],
                                R, v2own, relu=False)
                agin = dram.tile([128, 4, 780], BF, tag=f"agin{layer}")
                agout = dram.tile([2, 128, 4, 780], BF, tag=f"agout{layer}")
                nc.sync.dma_start(agin[:], v2own[:])
                nc.gpsimd.collective_compute(
                    "AllGather", OP.bypass, replica_groups=PAIRS,
                    ins=[agin[:].opt()], outs=[agout[:].opt()])
                v2 = tr.tile([128, 8, 780], BF, tag="v2full")
                nc.sync.dma_start(v2[:, 0:4, :], agout[0, :, :, :])
                nc.sync.dma_start(v2[:, 4:8, :], agout[1, :, :, :])
                if taps and layer == 0:
                    nc.sync.dma_start(tap_io["tv2f"][:], v2[:])

                # cross AV (full-array, accumulating all 8 chunks per head)
                att2 = attp.tile([128, 6, R], BF, tag="att")
                m2 = tr.tile([128, 6, R], BF, tag="tbf")
                rrds2 = []
                for p in range(NPAIR):
                    dflat2 = None
                    if layer == 0:
                        dflat2 = smallp.tile([1, 2 * R], BF, tag="dflat",
                                             bufs=2, name=f"dflat2_{p}")
                    for half in range(2):
                        h = 2 * p + half
                        off = 64 * half
                        po = ps_w()
                        for j in range(NKCH):
                            e = e2s[p][j // 2][half]
                            blk = (j % 2) * 512
                            nc.tensor.matmul(
                                po[0:65, 0:R],
                                v2[:, j, h * 65:h * 65 + 65],
                                e[:, blk:blk + 512],
                                start=(j == 0), stop=(j == NKCH - 1))
                        nc.vector.tensor_copy(att2[off:off + 64, p, :],
                                              po[0:64, 0:R])
                        if layer == 0:
                            nc.vector.tensor_copy(
                                dflat2[0:1, half * R:(half + 1) * R],
                                po[64:65, 0:R])
                    if layer == 0:
                        rrds2.append(
                            recip_pair(dflat2, dent2[2 * p:2 * p + 2, :]))
                    else:
                        rrd = rr_tile()
                        nc.sync.dma_start(out=rrd[:],
                                          in_=dent2[2 * p:2 * p + 2, :])
                        rrds2.append(rrd)
                for p in range(NPAIR):
                    bps = bcast_pair(rrds2[p])
                    nc.vector.tensor_tensor(m2[:, p, :], att2[:, p, :],
                                            bps[:, 0:R], op=OP.mult)
                if taps and layer == 0:
                    nc.sync.dma_start(tap_io["tatt2"][:], att2[:])
                    nc.sync.dma_start(tap_io["tm2"][:], m2[:])

                # x2 = m2 @ Wo2 + bo2 + t
                res2 = resp.tile([128, 6, R], F32, tag="res")
                t_prev = res
                dense_fm(w_sb["wo2"], 6,
                         lambda i, c0, cw: m2[:, i, c0:c0 + cw], R,
                         lambda j, c0, cw, ps: nc.vector.scalar_tensor_tensor(
                             res2[:, j, c0:c0 + cw], ps[:, 0:cw],
                             cvec[:, 24 + j:25 + j], t_prev[:, j, c0:c0 + cw],
                             op0=OP.add, op1=OP.add))
                if taps and layer == 0:
                    nc.sync.dma_start(tap_io["tx2"][:], res2[:])
                ar2 = bn_start(res2)
                w2 = bn_finish(ar2, 48, 54)  # g2, b2
                t2bf = tr.tile([128, 6, R], BF, tag="tbf")
                for jh in range(6):
                    bn_apply(w2, res2, t2bf, jh)
                for jh in range(6):
                    bn_apply(w2, res2, res2, jh)
                if taps and layer == 0:
                    nc.sync.dma_start(tap_io["tt2"][:], res2[:])

                # FFN: x3 = t2 @ Wf + bf + t2
                res3 = resp.tile([128, 6, R], F32, tag="res")
                dense_fm(w_sb["wf"], 6,
                         lambda i, c0, cw: t2bf[:, i, c0:c0 + cw], R,
                         lambda j, c0, cw, ps: nc.vector.scalar_tensor_tensor(
                             res3[:, j, c0:c0 + cw], ps[:, 0:cw],
                             cvec[:, 30 + j:31 + j], res2[:, j, c0:c0 + cw],
                             op0=OP.add, op1=OP.add))

                if layer < NLAYERS - 1:
                    # AllGather pre-BN x3 (bf16) concurrently with stats AR
                    xcast = tr.tile([128, 6, R], BF, tag="xcast")
                    for jh in range(6):
                        nc.vector.tensor_copy(xcast[:, jh, :], res3[:, jh, :])
                    xagin = dram.tile([128, 6, R], BF, tag="xagin")
                    xagout = dram.tile([2, 128, 6, R], BF, tag="xagout")
                    nc.sync.dma_start(xagin[:], xcast[:])
                    nc.gpsimd.collective_compute(
                        "AllGather", OP.bypass, replica_groups=PAIRS,
                        ins=[xagin[:].opt()], outs=[xagout[:].opt()])
                    ar3 = bn_start(res3)
                    nc.sync.dma_start(xin[:, :, 0:R], xagout[0, :, :, :])
                    nc.sync.dma_start(xin[:, :, R:S], xagout[1, :, :, :])
                    w3 = bn_finish(ar3, 48, 54)
                    # next-layer inputs: xin (all rows bf16), xq (own bf16),
                    # xo (own f32) -- all post-BN
                    xnew = tr.tile([128, 6, R], BF, tag="xcast")
                    for jh in range(6):
                        bn_apply(w3, res3, xnew, jh)
                        nc.vector.tensor_scalar(
                            xo[:, jh, :], res3[:, jh, :],
                            w3[:, 0 + jh:1 + jh], w3[:, 6 + jh:7 + jh],
                            op0=OP.mult, op1=OP.add)
                    for jh in range(6):
                        nc.vector.tensor_scalar(
                            xin[:, jh, :], xin[:, jh, :],
                            w3[:, 0 + jh:1 + jh], w3[:, 6 + jh:7 + jh],
                            op0=OP.mult, op1=OP.add)
                    if taps and layer == 0:
                        nc.sync.dma_start(tap_io["tout1"][:], xo[:])
                    xq_cur = xnew[:, 0:2, :]
                else:
                    ar3 = bn_start(res3)
                    w3 = bn_finish(ar3, 48, 54)
                    for jh in range(6):
                        bn_apply(w3, res3, res3, jh)
                    res_final = res3

            nc.sync.dma_start(out_io[:], res_final[:])

    nc.compile()
    return nc


def _host_prepare(inputs):
    x = np.asarray(inputs["x"])
    encod = np.asarray(inputs["encod"], np.float32)
    embed = np.asarray(inputs["embed"], np.float32)
    emb = embed[x.astype(np.int64)]
    im0 = 2.0 * emb + _pos_encoding()[None]  # [B,S,D] f32

    wq, wk, wv = (np.asarray(inputs[k], np.float32) for k in ("Wq", "Wk", "Wv"))
    wq2, wk2 = (np.asarray(inputs[k], np.float32) for k in ("Wq2", "Wk2"))
    wv2, wo2, wf = (np.asarray(inputs[k], np.float32) for k in ("Wv2", "Wo2", "Wf"))
    w_np = {nm: _bf16(_wchunk(w)) for nm, w in
            [("wq", wq), ("wk", wk), ("wv", wv), ("wq2", wq2), ("wk2", wk2),
             ("wv2", wv2), ("wo2", wo2), ("wf", wf)]}
    cvec = np.concatenate(
        [_col(np.asarray(inputs[k], np.float32)) for k in
         ("bq", "bk", "bq2", "bk2", "bo2", "bf", "g1", "b1", "g2", "b2")],
        axis=1).astype(np.float32)
    brow = np.concatenate([np.asarray(inputs["bv"], np.float32),
                           np.asarray(inputs["bv2"], np.float32)])[None, :].astype(np.float32)
    sel = np.zeros((2, 128), np.float32)
    sel[0, 0:64] = 1.0
    sel[1, 64:128] = 1.0

    in_maps = []
    for c in range(NC):
        b_, r_ = c // 2, c % 2
        rows = slice(r_ * R, (r_ + 1) * R)
        m = dict(w_np)
        m["cvec"] = cvec
        m["brow"] = brow
        m["sel"] = _bf16(sel)
        m["xin"] = _bf16(_fm(im0[b_]))
        m["xq"] = _bf16(_fm(im0[b_][rows, 0:256]))
        m["xo"] = _fm(im0[b_][rows]).astype(np.float32)
        m["encq"] = _bf16(_fm(encod[b_][rows, 0:384]))
        m["enck"] = _bf16(_fm(encod[b_][:, 384:768]))
        in_maps.append(m)
    return in_maps


def _gather(results):
    out = np.zeros((B, S, D), np.float32)
    for c in range(NC):
        b_, r_ = c // 2, c % 2
        a = results[c]["out"]  # [128, 6, R]
        out[b_, r_ * R:(r_ + 1) * R] = a.transpose(1, 0, 2).reshape(D, R).T
    return out


def kernel(**inputs) -> np.ndarray:
    from concourse.bass_utils import run_bass_kernel_spmd

    if "nc" not in _CACHE:
        _CACHE["nc"] = _build()
    nc = _CACHE["nc"]
    in_maps = _host_prepare(inputs)
    res = run_bass_kernel_spmd(nc, in_maps, core_ids=list(range(NC)))
    return _gather(res.results)


# revision 31
# speedup vs baseline: 1.3421x; 1.0046x over previous
"""Trainium2 Bass kernel for nn_Decoder (dense transformer decoder, 2 layers).

Sharding (8 cores): core c = 2*b + r handles batch b, query-row half r.
- Attention (scores/softmax/AV, all heads) is split by query rows.
- K/V projections are computed for all rows (duplicated within the pair).
- Cross-attention V2 is computed for own rows then pair-AllGathered.
- BatchNorm statistics are 8-rank AllReduced (sums over all B*S rows).
- Layer boundary: pair-AllGather of the new input_multi halves.

v2 rewrite vs baseline:
- Scores matmuls are 64-contraction row-tiled (two heads run concurrently on
  PE half-arrays, uniform (64,128) tile mode within self-attention).
- Scores land in PSUM as bf16: one [128,2048] bank-pair holds a head-pair x
  2 key-chunks, consumed by a single EXP instruction.
- AV is row-tiled over kpos halves into two accumulator banks (P/Q) with the
  V-aug ones column producing denominators in row 64.
- Softmax denominators: DVE adds into a [12,512] tile, one batched
  reciprocal, a rearrange-DMA into [2, 6*512], then one tiny selector-matmul
  per head pair broadcasts 1/den across 128 partitions in PSUM.
- Cross-attention scores+exp depend only on encod, so they are issued to
  cover the BN1 AllReduce and the V2 AllGather. Cross denominators are
  layer-invariant and cached from layer 0.
- Layer boundary: AllGather of the pre-BN FFN residual runs concurrently
  with the BN-stats AllReduce; the affine is applied locally afterwards.

Layout: activations are feature-major ("X^T", [feat, token]) stored as
[128, chunk, tok] SBUF tiles (feature f = 128*chunk + partition).
V / V2 are token-major [tok, head*65] with a ones column appended per head
(V-aug) so softmax denominators fall out of the AV matmul as row 64.
All matmuls are bf16 x bf16; the residual stream and BN statistics are f32.
"""
import numpy as np
import ml_dtypes

B, S, D, H, VOCAB, NLAYERS = 4, 1024, 768, 12, 32000, 2
HD = D // H          # 64
R = S // 2           # 512 own rows per core
NC = 8
SCALE1 = 1.0 / float(np.sqrt(D))
SCALE2 = 1.0 / float(np.sqrt(HD))
INV_N = 1.0 / (B * S)
NKCH = S // 128      # 8 key chunks
NPAIR = H // 2       # 6 head pairs

BF = None
F32 = None

_CACHE = {}


def _pos_encoding():
    p = np.arange(S, dtype=np.float32)[:, None]
    i = np.arange(D // 2, dtype=np.float32)[None, :]
    ang = p / np.power(10000.0, 2.0 * i / D)
    return np.stack([np.sin(ang), np.cos(ang)], axis=-1).reshape(S, D).astype(np.float32)


def _fm(a):
    """[tok, feat] -> feature-major chunked [128, nchunk, tok]."""
    t, f = a.shape
    return np.ascontiguousarray(a.T.reshape(f // 128, 128, t).transpose(1, 0, 2))


def _wchunk(w):
    """[in, out] weight -> [128, nin, out] (stationary chunks)."""
    i, o = w.shape
    return np.ascontiguousarray(w.reshape(i // 128, 128, o).transpose(1, 0, 2))


def _col(v):
    """[768] -> [128, 6] feature-major columns."""
    return np.ascontiguousarray(v.reshape(6, 128).T)


def _bf16(a):
    return np.asarray(a, np.float32).astype(ml_dtypes.bfloat16)


def _build(taps=False):
    import concourse.bass as bass
    import concourse.mybir as mybir
    import concourse.tile as tile
    from concourse import bacc

    global BF, F32
    BF = mybir.dt.bfloat16
    F32 = mybir.dt.float32
    AF = mybir.ActivationFunctionType
    OP = mybir.AluOpType

    nc = bacc.Bacc(None, target_bir_lowering=False, debug=False)

    # ---- I/O ----
    xin_io = nc.dram_tensor("xin", [128, 6, S], BF, kind="ExternalInput")
    xq_io = nc.dram_tensor("xq", [128, 2, R], BF, kind="ExternalInput")
    xo_io = nc.dram_tensor("xo", [128, 6, R], F32, kind="ExternalInput")
    encq_io = nc.dram_tensor("encq", [128, 3, R], BF, kind="ExternalInput")
    enck_io = nc.dram_tensor("enck", [128, 3, S], BF, kind="ExternalInput")
    w_io = {}
    for nm, nin in [("wq", 2), ("wk", 2), ("wv", 2), ("wq2", 3), ("wk2", 3),
                    ("wv2", 6), ("wo2", 6), ("wf", 6)]:
        w_io[nm] = nc.dram_tensor(nm, [128, nin, D], BF, kind="ExternalInput")
    cvec_io = nc.dram_tensor("cvec", [128, 60], F32, kind="ExternalInput")
    brow_io = nc.dram_tensor("brow", [1, 2 * D], F32, kind="ExternalInput")
    sel_io = nc.dram_tensor("sel", [2, 128], BF, kind="ExternalInput")
    out_io = nc.dram_tensor("out", [128, 6, R], F32, kind="ExternalOutput")
    tap_io = {}
    if taps:
        for nm, shp, dt_ in [
            ("tq2", [128, 6, R], "bf"), ("tk2", [128, 6, S], "bf"),
            ("tqt", [128, 6, R], "bf"), ("tkt", [128, 6, S], "bf"),
            ("tvt", [128, 8, 780], "bf"), ("te0", [128, 1024], "bf"),
            ("tatt", [128, 6, R], "bf"), ("tden", [12, R], "bf"),
            ("tx1", [128, 6, R], "f"), ("tt", [128, 6, R], "f"),
            ("tv2f", [128, 8, 780], "bf"), ("tatt2", [128, 6, R], "bf"),
            ("tm2", [128, 6, R], "bf"),
            ("tx2", [128, 6, R], "f"), ("tt2", [128, 6, R], "f"),
            ("tout1", [128, 6, R], "f"),
        ]:
            tap_io[nm] = nc.dram_tensor(nm, shp, BF if dt_ == "bf" else F32,
                                        kind="ExternalOutput")

    PAIRS = [[0, 1], [2, 3], [4, 5], [6, 7]]
    ALL8 = [list(range(NC))]

    with tile.TileContext(nc) as tc:
        with (
            tc.tile_pool(name="pp", bufs=1) as pp,
            tc.tile_pool(name="trans", bufs=1) as tr,
            tc.tile_pool(name="resp", bufs=2) as resp,
            tc.tile_pool(name="attp", bufs=1) as attp,
            tc.tile_pool(name="expp", bufs=10) as expp,
            tc.tile_pool(name="smallp", bufs=1) as smallp,
            tc.tile_pool(name="ps", bufs=1, space="PSUM") as psp,
            tc.tile_pool(name="dram", bufs=1, space="DRAM") as dram,
        ):
            _psn = [0]

            def ps_s():
                # scores staging: [128, 1024] f32 = 2 banks, double buffered
                _psn[0] += 1
                return psp.tile([128, 1024], F32, tag="s", bufs=2,
                                name=f"ps_s{_psn[0]}")

            def ps_w():
                # work psum: AV accumulators / dense outputs / broadcasts
                _psn[0] += 1
                return psp.tile([128, 512], F32, tag="w", bufs=4,
                                name=f"ps_w{_psn[0]}")

            # ---- persistent SBUF ----
            cvec = pp.tile([128, 60], F32, name="sb_cvec")
            nc.sync.dma_start(cvec[:], cvec_io[:])
            # preamble-critical tensors first (parked in layer-scratch tags)
            encq_t = tr.tile([128, 6, R], BF, tag="qbf")
            encq = encq_t[:, 0:3, :]
            nc.sync.dma_start(encq, encq_io[:])
            enck_t = tr.tile([128, 6, S], BF, tag="kbf")
            enck = enck_t[:, 0:3, :]
            nc.sync.dma_start(enck, enck_io[:])
            w_sb = {}
            for nm in ("wq2", "wk2", "wq", "wk", "wv"):
                t_io = w_io[nm]
                w_sb[nm] = pp.tile(list(t_io.shape), BF, name=f"sb_{nm}")
                nc.sync.dma_start(w_sb[nm][:], t_io[:])
            xq1 = pp.tile([128, 2, R], BF, name="sb_xq1")
            nc.sync.dma_start(xq1[:], xq_io[:])
            xin = pp.tile([128, 6, S], BF, name="sb_xin")
            nc.sync.dma_start(xin[:], xin_io[:])
            for nm in ("wv2", "wo2", "wf"):
                t_io = w_io[nm]
                w_sb[nm] = pp.tile(list(t_io.shape), BF, name=f"sb_{nm}")
                nc.sync.dma_start(w_sb[nm][:], t_io[:])
            xo = pp.tile([128, 6, R], F32, name="sb_xo")
            nc.sync.dma_start(xo[:], xo_io[:])

            zero_col = pp.tile([128, 1], F32, name="sb_zero")
            nc.vector.memset(zero_col[:], 0.0)
            eps_col = pp.tile([128, 1], F32, name="sb_eps")
            nc.vector.memset(eps_col[:], 1e-5)

            # selector for denominator broadcast: out[p] = rhs[p//64]
            sel = pp.tile([2, 128], BF, name="sb_sel")
            nc.sync.dma_start(sel[:], sel_io[:])

            # bias broadcast rows for token-major V / V2 evictions
            bias_bc = []
            for bi in range(2):
                t = pp.tile([128, D], F32, name=f"sb_biasbc{bi}")
                nc.sync.dma_start(
                    out=t[:, :],
                    in_=brow_io[0:1, bi * D:(bi + 1) * D].broadcast_to([128, D]))
                bias_bc.append(t)

            # cross-attn 1/denominators are layer-invariant; filled in layer 0
            dent2 = pp.tile([12, R], BF, name="sb_dent2")

            # ---- helpers ----
            def dense_fm(w, nin, rhs_fn, ncols, evict_fn):
                """out^T[128j+p, col] accumulation over nin input chunks."""
                for j in range(6):
                    for c0 in range(0, ncols, 512):
                        cw = min(512, ncols - c0)
                        ps = ps_w()
                        for i in range(nin):
                            nc.tensor.matmul(
                                ps[:, 0:cw],
                                w[:, i, j * 128:(j + 1) * 128],
                                rhs_fn(i, c0, cw),
                                start=(i == 0), stop=(i == nin - 1))
                        evict_fn(j, c0, cw, ps)

            def relu_evict(dst, base):
                """DVE eviction: relu(psum + bias_col)."""
                def f(j, c0, cw, ps):
                    nc.vector.tensor_scalar(
                        dst[:, j, c0:c0 + cw], ps[:, 0:cw],
                        cvec[:, base + j:base + j + 1], 0.0,
                        op0=OP.add, op1=OP.max)
                return f

            def iden_evict(dst, base):
                """DVE eviction: psum + bias_col."""
                def f(j, c0, cw, ps):
                    nc.vector.tensor_scalar_add(
                        dst[:, j, c0:c0 + cw], ps[:, 0:cw],
                        cvec[:, base + j:base + j + 1])
                return f

            def tokenmajor_vaug(w, bias_bc_t, x_lhs_fn, ntok, dst, relu):
                """V / V2 production: [tok, 12*65] with aug ones columns."""
                ntch = ntok // 128
                for tch in range(ntch):
                    nc.vector.memset(
                        dst[:, tch, :].rearrange("p (h k) -> p h k", k=65)[:, :, 64:65],
                        1.0)
                    for half in range(2):
                        ps = ps_w()
                        nin = w.shape[1]
                        for i in range(nin):
                            nc.tensor.matmul(
                                ps[:, 0:384],
                                x_lhs_fn(i, tch),
                                w[:, i, half * 384:(half + 1) * 384],
                                start=(i == 0), stop=(i == nin - 1))
                        nc.vector.tensor_tensor(
                            ps[:, 0:384], ps[:, 0:384],
                            bias_bc_t[:, half * 384:(half + 1) * 384], op=OP.add)
                        src = ps[:, 0:384].rearrange("p (h k) -> p h k", k=64)
                        dstap = dst[:, tch, :].rearrange(
                            "p (h k) -> p h k", k=65)[:, half * 6:(half + 1) * 6, 0:64]
                        if relu:
                            nc.scalar.activation(dstap, src, AF.Relu, bias=zero_col[:])
                        else:
                            nc.scalar.copy(dstap, src)

            def scores_exp(q_t, k_t, scale, p, jp, etap=None, spill=None):
                """Head-pair p, key-chunk-pair jp -> (eA, eB) [128,1024] bf16.

                Per head: columns [chunk j, chunk j+1] with j = 2*jp.
                A = head 2p (features in partitions 0:64), B = head 2p+1.
                Scores run row-tiled: A on PE rows 0:64, B on rows 64:128,
                concurrently, into separate PSUM bank pairs.
                """
                sab = [ps_s(), ps_s()]
                j0 = 2 * jp
                for dj in range(2):
                    j = j0 + dj
                    for half in range(2):
                        off = 64 * half
                        nc.tensor.matmul(
                            sab[half][:, dj * 512:(dj + 1) * 512],
                            k_t[off:off + 64, p, j * 128:(j + 1) * 128],
                            q_t[off:off + 64, p, :],
                            start=True, stop=True)
                eab = []
                for half in range(2):
                    if spill is None:
                        e = expp.tile([128, 1024], BF, tag="e",
                                      name=f"e{_psn[0]}_{half}")
                    else:
                        e = expp.tile([128, 1024], BF, tag="es", bufs=3,
                                      name=f"es{_psn[0]}_{half}")
                    nc.scalar.activation(e[:], sab[half][:], AF.Exp,
                                         bias=zero_col[:], scale=scale)
                    if spill is not None:
                        nc.sync.dma_start(spill[:, 2 * jp + half, :], e[:])
                    eab.append(e)
                if etap is not None:
                    nc.sync.dma_start(etap[:], eab[0][:])
                return eab

            _rrn = [0]

            def rr_tile():
                _rrn[0] += 1
                return smallp.tile([2, R], BF, tag="rrep", bufs=6,
                                   name=f"rr{_rrn[0]}")

            def recip_pair(dflat_t, dent2_dst):
                """dflat [1, 2R] (den A | den B) -> rrd [2, R] = 1/den."""
                rrd = rr_tile()
                nc.sync.dma_start(out=rrd[0:1, :], in_=dflat_t[0:1, 0:R])
                nc.sync.dma_start(out=rrd[1:2, :], in_=dflat_t[0:1, R:2 * R])
                with nc.allow_low_precision(reason="softmax denom recip, bf16"):
                    nc.vector.reciprocal(rrd[:], rrd[:])
                if dent2_dst is not None:
                    nc.sync.dma_start(out=dent2_dst, in_=rrd[:])
                return rrd

            def bcast_pair(rrd):
                """Broadcast rrd [2, R] across partitions: out[p] = rrd[p//64]."""
                ps = ps_w()
                nc.tensor.matmul(ps[:, 0:R], sel[:, :], rrd[:],
                                 start=True, stop=True)
                return ps


            _rrn = [0]

            def rr_tile():
                _rrn[0] += 1
                return smallp.tile([2, R], BF, tag="rrep", bufs=6,
                                   name=f"rr{_rrn[0]}")

            def recip_pair(dflat_t, dent2_dst):
                """dflat [1, 2R] (den A | den B) -> rrd [2, R] = 1/den."""
                rrd = rr_tile()
                nc.sync.dma_start(out=rrd[0:1, :], in_=dflat_t[0:1, 0:R])
                nc.sync.dma_start(out=rrd[1:2, :], in_=dflat_t[0:1, R:2 * R])
                with nc.allow_low_precision(reason="softmax denom recip, bf16"):
                    nc.vector.reciprocal(rrd[:], rrd[:])
                if dent2_dst is not None:
                    nc.sync.dma_start(out=dent2_dst, in_=rrd[:])
                return rrd

            def bcast_pair(rrd):
                """Broadcast rrd [2, R] across partitions: out[p] = rrd[p//64]."""
                ps = ps_w()
                nc.tensor.matmul(ps[:, 0:R], sel[:, :], rrd[:],
                                 start=True, stop=True)
                return ps

            bn_idx = [0]

            def bn_launch(stats):
                """DMA stats to DRAM + AllReduce launch. Returns arout."""
                i = bn_idx[0]
                bn_idx[0] += 1
                arin = dram.tile([128, 12], F32, tag=f"arin{i}")
                arout = dram.tile([128, 12], F32, tag=f"arout{i}",
                                  addr_space="Shared")
                nc.sync.dma_start(arin[:], stats[:])
                nc.gpsimd.collective_compute(
                    "AllReduce", OP.add, replica_groups=ALL8,
                    ins=[arin[:].opt()], outs=[arout[:].opt()])
                return arout

            def bn_start(res):
                """Stats of res [128,6,R] + AllReduce launch. Returns arout."""
                stats = smallp.tile([128, 12], F32, tag=f"stats{bn_idx[0]}",
                                    name=f"stats{bn_idx[0]}")
                for jh in range(6):
                    nc.vector.reduce_sum(stats[:, jh:jh + 1], res[:, jh, :],
                                         axis=mybir.AxisListType.X)
                for jh in range(6):
                    sq = ps_w()
                    nc.scalar.activation(sq[:, 0:R], res[:, jh, :], AF.Square,
                                         bias=zero_col[:],
                                         accum_out=stats[:, 6 + jh:7 + jh])
                return bn_launch(stats)

            def bn_finish(arout, gbase, bbase):
                """AR result -> per-feature scale (w[:,0:6]) / shift (w[:,6:12])."""
                i = bn_idx[0] - 1
                g = smallp.tile([128, 12], F32, tag=f"g{i}")
                nc.sync.dma_start(g[:], arout[:])
                w = smallp.tile([128, 18], F32, tag=f"bnw{i}")
                # mu = sum/N ; msq = sumsq/N
                nc.vector.tensor_scalar_mul(w[:, 12:18], g[:, 0:6], INV_N)
                nc.vector.tensor_scalar_mul(w[:, 6:12], g[:, 6:12], INV_N)
                # var = msq - mu^2
                nc.vector.tensor_tensor(w[:, 0:6], w[:, 12:18], w[:, 12:18],
                                        op=OP.mult)
                nc.vector.tensor_tensor(w[:, 0:6], w[:, 6:12], w[:, 0:6],
                                        op=OP.subtract)
                # std = sqrt(var + eps); rstd = 1/std
                nc.scalar.activation(w[:, 6:12], w[:, 0:6], AF.Sqrt, bias=eps_col[:])
                nc.vector.reciprocal(w[:, 0:6], w[:, 6:12])
                # scale = rstd*gamma ; shift = beta - mu*scale
                nc.vector.tensor_tensor(w[:, 0:6], w[:, 0:6],
                                        cvec[:, gbase:gbase + 6], op=OP.mult)
                nc.vector.tensor_tensor(w[:, 6:12], w[:, 12:18], w[:, 0:6],
                                        op=OP.mult)
                nc.vector.tensor_tensor(w[:, 6:12], cvec[:, bbase:bbase + 6],
                                        w[:, 6:12], op=OP.subtract)
                return w

            def bn_apply(w, src, dst, jh):
                """dst[:, jh, :] = src[:, jh, :] * scale + shift."""
                nc.vector.tensor_scalar(dst[:, jh, :], src[:, jh, :],
                                        w[:, 0 + jh:1 + jh], w[:, 6 + jh:7 + jh],
                                        op0=OP.mult, op1=OP.add)

            # ---- preamble: cross-attn Q2 (own rows), K2 (all rows) ----
            # computed once into DRAM; reloaded into qt/kt scratch per layer
            q2d = dram.tile([128, 6, R], BF, tag="q2d")
            k2d = dram.tile([128, 6, S], BF, tag="k2d")
            # spilled cross-attn exp(scores): [pair, 2*jp+half, qpos]
            e2d = dram.tile([128, 6, 8, 1024], BF, tag="e2d")
            q2tmp = attp.tile([128, 6, R], BF, tag="att", name="q2tmp")
            dense_fm(w_sb["wq2"], 3, lambda i, c0, cw: encq[:, i, c0:c0 + cw], R,
                     iden_evict(q2tmp, 12))
            nc.sync.dma_start(q2d[:], q2tmp[:])
            for kh in range(2):
                k2tmp = attp.tile([128, 6, R], BF, tag="att",
                                  name=f"k2tmp{kh}")
                dense_fm(w_sb["wk2"], 3,
                         lambda i, c0, cw, kh=kh:
                         enck[:, i, kh * R + c0:kh * R + c0 + cw], R,
                         iden_evict(k2tmp, 18))
                nc.sync.dma_start(k2d[:, :, kh * R:(kh + 1) * R], k2tmp[:])
            if taps:
                nc.sync.dma_start(tap_io["tq2"][:], q2d[:])
                nc.sync.dma_start(tap_io["tk2"][:], k2d[:])

            # ---- layers ----
            xq_cur = xq1
            res_final = None
            for layer in range(NLAYERS):
                # Q (own rows), K (all rows) feature-major, with relu
                qt = tr.tile([128, 6, R], BF, tag="qbf")
                kt = tr.tile([128, 6, S], BF, tag="kbf")
                dense_fm(w_sb["wq"], 2,
                         lambda i, c0, cw: xq_cur[:, i, c0:c0 + cw], R,
                         relu_evict(qt, 0))
                dense_fm(w_sb["wk"], 2,
                         lambda i, c0, cw: xin[:, 2 + i, c0:c0 + cw], S,
                         relu_evict(kt, 6))
                # V token-major (all rows), relu, aug
                vt = tr.tile([128, 8, 780], BF, tag="vbf")
                tokenmajor_vaug(w_sb["wv"], bias_bc[0],
                                lambda i, tch: xin[:, 4 + i, tch * 128:(tch + 1) * 128],
                                S, vt, relu=True)
                if taps and layer == 0:
                    nc.sync.dma_start(tap_io["tqt"][:], qt[:])
                    nc.sync.dma_start(tap_io["tkt"][:], kt[:])
                    nc.sync.dma_start(tap_io["tvt"][:], vt[:])

                # ---- self attention (row-tiled scores, plain AV) ----
                att = attp.tile([128, 6, R], BF, tag="att")
                res = resp.tile([128, 6, R], F32, tag="res")
                stats = smallp.tile([128, 12], F32, tag=f"stats_a{layer}")
                rrds = []
                for p in range(NPAIR):
                    dflat = smallp.tile([1, 2 * R], BF, tag="dflat", bufs=2,
                                        name=f"dflat{layer}_{p}")
                    po = [ps_w(), ps_w()]
                    for jp in range(4):
                        eab = scores_exp(
                            qt, kt, SCALE1, p, jp,
                            etap=tap_io.get("te0") if taps and layer == 0
                            and p == 0 and jp == 0 else None)
                        j0 = 2 * jp
                        for half in range(2):          # head A / head B
                            h = 2 * p + half
                            for dj in range(2):        # chunk j0+dj
                                j = j0 + dj
                                nc.tensor.matmul(
                                    po[half][0:65, 0:R],
                                    vt[:, j, h * 65:h * 65 + 65],
                                    eab[half][:, dj * 512:(dj + 1) * 512],
                                    start=(jp == 0 and dj == 0),
                                    stop=(jp == 3 and dj == 1))
                    for half in range(2):
                        h = 2 * p + half
                        off = 64 * half
                        nc.vector.tensor_copy(att[off:off + 64, p, :],
                                              po[half][0:64, 0:R])
                        nc.vector.tensor_copy(dflat[0:1, half * R:(half + 1) * R],
                                              po[half][64:65, 0:R])
                    rrds.append(recip_pair(dflat, None))
                # tail: broadcasts fire with all reciprocals ready (no PE stall)
                for p in range(NPAIR):
                    bps = bcast_pair(rrds[p])
                    nc.vector.tensor_tensor(res[:, p, :], att[:, p, :],
                                            bps[:, 0:R], op=OP.mult)
                    nc.vector.tensor_tensor(res[:, p, :], res[:, p, :],
                                            xo[:, p, :], op=OP.add)
                    nc.vector.reduce_sum(stats[:, p:p + 1], res[:, p, :],
                                         axis=mybir.AxisListType.X)
                    sqp = ps_w()
                    nc.scalar.activation(sqp[:, 0:R], res[:, p, :], AF.Square,
                                         bias=zero_col[:],
                                         accum_out=stats[:, 6 + p:7 + p])
                if taps and layer == 0:
                    nc.sync.dma_start(tap_io["tatt"][:], att[:])
                    nc.sync.dma_start(tap_io["tx1"][:], res[:])

                # BN1 AllReduce, covered by cross-attn scores below
                ar1 = bn_launch(stats)

                # cross-attn scores+exp depend only on encod: computed in
                # layer 0 (covering the BN1 AR + V2 AllGather windows) and
                # spilled to DRAM; layer 1 only prefetches.
                if layer == 0:
                    q2t = tr.tile([128, 6, R], BF, tag="qbf", name="q2t")
                    nc.sync.dma_start(q2t[:], q2d[:])
                    k2t = tr.tile([128, 6, S], BF, tag="kbf", name="k2t")
                    nc.sync.dma_start(k2t[:], k2d[:])
                    for p in range(NPAIR):
                        for jp in range(4):
                            scores_exp(q2t, k2t, SCALE2, p, jp,
                                       spill=e2d[:, p, :, :])

                w1 = bn_finish(ar1, 36, 42)  # g1, b1
                tbf = tr.tile([128, 6, R], BF, tag="tbf")
                for jh in range(6):
                    bn_apply(w1, res, tbf, jh)   # t in bf16 (for V2 proj)
                for jh in range(6):
                    bn_apply(w1, res, res, jh)   # t in f32 (residual)
                if taps and layer == 0:
                    nc.sync.dma_start(tap_io["tt"][:], res[:])

                # V2 own rows from t, then pair AllGather
                v2own = tr.tile([128, 4, 780], BF, tag="v2own")
                tokenmajor_vaug(w_sb["wv2"], bias_bc[1],
                                lambda i, tch: tbf[:, i, tch * 128:(tch + 1) * 128],
                                R, v2own, relu=False)
                agin = dram.tile([128, 4, 780], BF, tag=f"agin{layer}")
                agout = dram.tile([2, 128, 4, 780], BF, tag=f"agout{layer}")
                nc.sync.dma_start(agin[:], v2own[:])
                nc.gpsimd.collective_compute(
                    "AllGather", OP.bypass, replica_groups=PAIRS,
                    ins=[agin[:].opt()], outs=[agout[:].opt()])
                v2 = tr.tile([128, 8, 780], BF, tag="v2full")
                nc.sync.dma_start(v2[:, 0:4, :], agout[0, :, :, :])
                nc.sync.dma_start(v2[:, 4:8, :], agout[1, :, :, :])
                if taps and layer == 0:
                    nc.sync.dma_start(tap_io["tv2f"][:], v2[:])

                # cross AV (full-array, accumulating all 8 chunks per head)
                att2 = attp.tile([128, 6, R], BF, tag="att")
                m2 = tr.tile([128, 6, R], BF, tag="tbf")
                rrds2 = []
                for p in range(NPAIR):
                    dflat2 = None
                    if layer == 0:
                        dflat2 = smallp.tile([1, 2 * R], BF, tag="dflat",
                                             bufs=2, name=f"dflat2_{p}")
                    epair = []
                    for jp in range(4):
                        for half in range(2):
                            ep = expp.tile([128, 1024], BF, tag="e",
                                           name=f"ec{layer}_{p}_{jp}_{half}")
                            nc.sync.dma_start(ep[:],
                                              e2d[:, p, 2 * jp + half, :])
                            epair.append(ep)
                    for half in range(2):
                        h = 2 * p + half
                        off = 64 * half
                        po = ps_w()
                        for j in range(NKCH):
                            e = epair[2 * (j // 2) + half]
                            blk = (j % 2) * 512
                            nc.tensor.matmul(
                                po[0:65, 0:R],
                                v2[:, j, h * 65:h * 65 + 65],
                                e[:, blk:blk + 512],
                                start=(j == 0), stop=(j == NKCH - 1))
                        nc.vector.tensor_copy(att2[off:off + 64, p, :],
                                              po[0:64, 0:R])
                        if layer == 0:
                            nc.vector.tensor_copy(
                                dflat2[0:1, half * R:(half + 1) * R],
                                po[64:65, 0:R])
                    if layer == 0:
                        rrds2.append(
                            recip_pair(dflat2, dent2[2 * p:2 * p + 2, :]))
                    else:
                        rrd = rr_tile()
                        nc.sync.dma_start(out=rrd[:],
                                          in_=dent2[2 * p:2 * p + 2, :])
                        rrds2.append(rrd)
                for p in range(NPAIR):
                    bps = bcast_pair(rrds2[p])
                    nc.vector.tensor_tensor(m2[:, p, :], att2[:, p, :],
                                            bps[:, 0:R], op=OP.mult)
                if taps and layer == 0:
                    nc.sync.dma_start(tap_io["tatt2"][:], att2[:])
                    nc.sync.dma_start(tap_io["tm2"][:], m2[:])

                # x2 = m2 @ Wo2 + bo2 + t
                res2 = resp.tile([128, 6, R], F32, tag="res")
                t_prev = res
                dense_fm(w_sb["wo2"], 6,
                         lambda i, c0, cw: m2[:, i, c0:c0 + cw], R,
                         lambda j, c0, cw, ps: nc.vector.scalar_tensor_tensor(
                             res2[:, j, c0:c0 + cw], ps[:, 0:cw],
                             cvec[:, 24 + j:25 + j], t_prev[:, j, c0:c0 + cw],
                             op0=OP.add, op1=OP.add))
                if taps and layer == 0:
                    nc.sync.dma_start(tap_io["tx2"][:], res2[:])
                ar2 = bn_start(res2)
                w2 = bn_finish(ar2, 48, 54)  # g2, b2
                t2bf = tr.tile([128, 6, R], BF, tag="tbf")
                for jh in range(6):
                    bn_apply(w2, res2, t2bf, jh)
                for jh in range(6):
                    bn_apply(w2, res2, res2, jh)
                if taps and layer == 0:
                    nc.sync.dma_start(tap_io["tt2"][:], res2[:])

                # FFN: x3 = t2 @ Wf + bf + t2
                res3 = resp.tile([128, 6, R], F32, tag="res")
                dense_fm(w_sb["wf"], 6,
                         lambda i, c0, cw: t2bf[:, i, c0:c0 + cw], R,
                         lambda j, c0, cw, ps: nc.vector.scalar_tensor_tensor(
                             res3[:, j, c0:c0 + cw], ps[:, 0:cw],
                             cvec[:, 30 + j:31 + j], res2[:, j, c0:c0 + cw],
                             op0=OP.add, op1=OP.add))

                if layer < NLAYERS - 1:
                    # AllGather pre-BN x3 (bf16) concurrently with stats AR
                    xcast = tr.tile([128, 6, R], BF, tag="xcast")
                    for jh in range(6):
                        nc.vector.tensor_copy(xcast[:, jh, :], res3[:, jh, :])
                    xagin = dram.tile([128, 6, R], BF, tag="xagin")
                    xagout = dram.tile([2, 128, 6, R], BF, tag="xagout")
                    nc.sync.dma_start(xagin[:], xcast[:])
                    nc.gpsimd.collective_compute(
                        "AllGather", OP.bypass, replica_groups=PAIRS,
                        ins=[xagin[:].opt()], outs=[xagout[:].opt()])
                    ar3 = bn_start(res3)
                    nc.sync.dma_start(xin[:, :, 0:R], xagout[0, :, :, :])
                    nc.sync.dma_start(xin[:, :, R:S], xagout[1, :, :, :])
                    w3 = bn_finish(ar3, 48, 54)
                    # next-layer inputs: xin (all rows bf16), xq (own bf16),
                    # xo (own f32) -- all post-BN
                    xnew = tr.tile([128, 6, R], BF, tag="xcast")
                    for jh in range(6):
                        bn_apply(w3, res3, xnew, jh)
                        nc.vector.tensor_scalar(
                            xo[:, jh, :], res3[:, jh, :],
                            w3[:, 0 + jh:1 + jh], w3[:, 6 + jh:7 + jh],
                            op0=OP.mult, op1=OP.add)
                    for jh in range(6):
                        nc.vector.tensor_scalar(
                            xin[:, jh, :], xin[:, jh, :],
                            w3[:, 0 + jh:1 + jh], w3[:, 6 + jh:7 + jh],
                            op0=OP.mult, op1=OP.add)
                    if taps and layer == 0:
                        nc.sync.dma_start(tap_io["tout1"][:], xo[:])
                    xq_cur = xnew[:, 0:2, :]
                else:
                    ar3 = bn_start(res3)
                    w3 = bn_finish(ar3, 48, 54)
                    for jh in range(6):
                        bn_apply(w3, res3, res3, jh)
                    res_final = res3

            nc.sync.dma_start(out_io[:], res_final[:])

    nc.compile()
    return nc


def _host_prepare(inputs):
    x = np.asarray(inputs["x"])
    encod = np.asarray(inputs["encod"], np.float32)
    embed = np.asarray(inputs["embed"], np.float32)
    emb = embed[x.astype(np.int64)]
    im0 = 2.0 * emb + _pos_encoding()[None]  # [B,S,D] f32

    wq, wk, wv = (np.asarray(inputs[k], np.float32) for k in ("Wq", "Wk", "Wv"))
    wq2, wk2 = (np.asarray(inputs[k], np.float32) for k in ("Wq2", "Wk2"))
    wv2, wo2, wf = (np.asarray(inputs[k], np.float32) for k in ("Wv2", "Wo2", "Wf"))
    w_np = {nm: _bf16(_wchunk(w)) for nm, w in
            [("wq", wq), ("wk", wk), ("wv", wv), ("wq2", wq2), ("wk2", wk2),
             ("wv2", wv2), ("wo2", wo2), ("wf", wf)]}
    cvec = np.concatenate(
        [_col(np.asarray(inputs[k], np.float32)) for k in
         ("bq", "bk", "bq2", "bk2", "bo2", "bf", "g1", "b1", "g2", "b2")],
        axis=1).astype(np.float32)
    brow = np.concatenate([np.asarray(inputs["bv"], np.float32),
                           np.asarray(inputs["bv2"], np.float32)])[None, :].astype(np.float32)
    sel = np.zeros((2, 128), np.float32)
    sel[0, 0:64] = 1.0
    sel[1, 64:128] = 1.0

    in_maps = []
    for c in range(NC):
        b_, r_ = c // 2, c % 2
        rows = slice(r_ * R, (r_ + 1) * R)
        m = dict(w_np)
        m["cvec"] = cvec
        m["brow"] = brow
        m["sel"] = _bf16(sel)
        m["xin"] = _bf16(_fm(im0[b_]))
        m["xq"] = _bf16(_fm(im0[b_][rows, 0:256]))
        m["xo"] = _fm(im0[b_][rows]).astype(np.float32)
        m["encq"] = _bf16(_fm(encod[b_][rows, 0:384]))
        m["enck"] = _bf16(_fm(encod[b_][:, 384:768]))
        in_maps.append(m)
    return in_maps


def _gather(results):
    out = np.zeros((B, S, D), np.float32)
    for c in range(NC):
        b_, r_ = c // 2, c % 2
        a = results[c]["out"]  # [128, 6, R]
        out[b_, r_ * R:(r_ + 1) * R] = a.transpose(1, 0, 2).reshape(D, R).T
    return out


def kernel(**inputs) -> np.ndarray:
    from concourse.bass_utils import run_bass_kernel_spmd

    if "nc" not in _CACHE:
        _CACHE["nc"] = _build()
    nc = _CACHE["nc"]
    in_maps = _host_prepare(inputs)
    res = run_bass_kernel_spmd(nc, in_maps, core_ids=list(range(NC)))
    return _gather(res.results)
